# revision 31
# baseline (speedup 1.0000x reference)
"""BidirectionalAttention Trainium2 Bass kernel — 8-core SPMD, v2.

Decomposition (same math as the verified baseline):
  q path : 1x1 conv (matmul) -> grouped conv1d k=3 -> conv1d k=3
  attn   : E = exp(q^T k); both softmaxes share one exp:
             attn_f + attn_b = E * (1/S0[n,m] + 1/S1[b,m])
             S0 = sum_b E  (batch softmax denom, axis=0)
             S1 = sum_n E  (row softmax denom, axis=1) -> two AllReduces
  fusion = value @ (attn_f+attn_b)^T scaled by gamma*mean(x_b), + x
  ConvTranspose2d(k=4,s=2,p=1) via the 4-subkernel parity decomposition,
  18-row output slabs with additive 2-row seams stitched on the host.

v2 performance changes vs the baseline:
  - K/Q/V in fp8e4 (K and V AllGathers halve; the attention branch output
    is scaled by gamma*mean(x) ~ 1e-3 so it tolerates fp8 easily).
    V is upcast to bf16 on the Scalar engine before the fusion matmuls.
  - E stays bf16 (DVE 2x perf mode requires 2-byte dtypes end-to-end).
  - One K AllGather for all 4 batches (was 4, each paying the ~15us ncfw
    floor).  CC-queue order: K-AG -> V-AG -> AR1 -> AR2, sized so each
    hides under local compute.
  - Phase C: one exp per m-tile, S1 via a single DVE tensor_reduce into a
    bf16 row (2x mode), S0 via an add tree split DVE/GpSimd by mt parity,
    1/S0 cached in bf16 for phase D (32 x 1KB/lane).
  - Phase D: A = (1/S0 + 1/S1) * E as ONE scalar_tensor_tensor per batch
    (replaces 4 tensor_scalars + a [128,2048] multiply), all operands bf16
    so DVE runs 2x.  Fusion matmuls then stream back-to-back to keep the
    PE warm (HAM throttling halved the baseline's matmul rate).
  - Coalesced DMAs: one const pack, one x pack, per-b V stages, one wco
    load, one output DMA per parity row.  Output returned in bf16.
"""

import numpy as np

B = 4
C = 256
H = 64
Wd = 64
HW = H * Wd            # 4096
CR = 32                # C // 8
NCORES = 8
NL = HW // NCORES      # 512 owned attention rows (n) per core
HL = H // NCORES       # 8 owned image rows per core
MT = HW // 128         # 32 m-tiles of 128
XW = NL + 4            # x slab width (n halo +-2 for the two k=3 convs)
Q2W = NL + 2           # q2 width (halo +-1 for conv2)
ROWW = 68              # fusion_pad row width: [0,1]=zero, 2..65 data, [66,67]=zero
OUTROWS = 2 * HL + 2   # 18 output rows per core (2-row overlaps, host-stitched)

# const-pack column offsets (bf16 elements)
OFF_WQ = 0             # [2, 256]
OFF_WV = 512           # [2, 256]
OFF_W1 = 1024          # [3, 2, 32]
OFF_MASK = 1216        # [516]
OFF_BVB = 1732         # [256]
OFF_W2 = 1988          # rows 0:32, [3, 64]
CPCOLS = 2180

_CACHE = {}


# ---------------------------------------------------------------------------
# device module
# ---------------------------------------------------------------------------
def build_module():
    from contextlib import ExitStack

    import concourse.bass as bass
    import concourse.mybir as mybir
    from concourse import bacc
    from concourse.tile import TileContext

    f32 = mybir.dt.float32
    bf16 = mybir.dt.bfloat16
    f8 = mybir.dt.float8e4
    AF = mybir.ActivationFunctionType
    OP = mybir.AluOpType
    AX = mybir.AxisListType

    nc = bacc.Bacc(num_devices=NCORES)
    RG = [list(range(NCORES))]

    # ---- parameters (per-core) -------------------------------------------
    cpack_p = nc.declare_dram_parameter("cpack", [128, CPCOLS], bf16, isOutput=False)
    fpack_p = nc.declare_dram_parameter("fpack", [128, 8], f32, isOutput=False)
    xpack_p = nc.declare_dram_parameter("xpack", [128, B, 2, XW], bf16, isOutput=False)
    wco_p = nc.declare_dram_parameter("wco", [32, 128, 128], bf16, isOutput=False)
    out_p = nc.declare_dram_parameter(
        "out", [B, C // 2, OUTROWS, 2 * Wd], bf16, isOutput=True
    )

    with TileContext(nc) as tc, ExitStack() as ctx:
        # ---- long-lived pools -------------------------------------------
        const = ctx.enter_context(tc.tile_pool(name="const", bufs=1))
        xpool = ctx.enter_context(tc.tile_pool(name="xp", bufs=1))
        qkv = ctx.enter_context(tc.tile_pool(name="qkv", bufs=1))
        fpool = ctx.enter_context(tc.tile_pool(name="fp", bufs=1))
        dram = ctx.enter_context(tc.tile_pool(name="dram", bufs=1, space="DRAM"))

        # ---- DRAM bounce buffers ----------------------------------------
        k_in = dram.tile([B, CR, NL], f8, tag="k_in", name="k_in")
        k_out = dram.tile(
            [NCORES, B, CR, NL], f8, tag="k_out", name="k_out"
        )
        v_in = dram.tile([B, NL, C], f8, tag="v_in", name="v_in")
        v_out = dram.tile(
            [NCORES, B, NL, C], f8, tag="v_out", name="v_out"
        )
        ar1_in = dram.tile([128, 64], f32, tag="ar1_in", name="ar1_in")
        ar1_out = dram.tile(
            [128, 64], f32, tag="ar1_out", name="ar1_out"
        )
        ar2_in = dram.tile([128, 72], f32, tag="ar2_in", name="ar2_in")
        ar2_out = dram.tile(
            [128, 72], f32, tag="ar2_out", name="ar2_out"
        )
        g_dram = dram.tile([1, B], f32, tag="g_dram", name="g_dram")
        warm_in = dram.tile([1, 4], f32, tag="warm_in", name="warm_in")
        warm_out = dram.tile([NCORES, 4], f32, tag="warm_out", name="warm_out")

        # warm-up rendezvous: absorb the cross-core NEFF-start skew under
        # phase A instead of paying it at the first real collective
        with tc.high_priority():
            nc.gpsimd.collective_compute(
                "AllGather", OP.bypass, replica_groups=RG,
                ins=[warm_in[:, :]], outs=[warm_out[:, :]],
            )

        # ---- persistent SBUF state --------------------------------------
        fpk = const.tile([128, 8], f32, tag="fpk", name="fpk")
        nc.sync.dma_start(out=fpk, in_=fpack_p[:, :])
        xt = xpool.tile([128, B, 2, XW], bf16, tag="xt", name="xt")
        nc.sync.dma_start(out=xt, in_=xpack_p[:, :, :, :])

        s1p = qkv.tile([128, 136], f32, tag="s1p", name="s1p")
        Q_all = qkv.tile([128, NL], f8, tag="Q", name="Q")
        K_all = qkv.tile([128, HW], f8, tag="K", name="K")
        r1a = qkv.tile([128, 64], bf16, tag="r1a", name="r1a")  # 1/S1, mt<16
        r1b = qkv.tile([128, 64], bf16, tag="r1b", name="r1b")  # 1/S1, mt>=16
        g_bcast = qkv.tile([128, B], f32, tag="gbc", name="gbc")
        a1o = qkv.tile([128, 64], f32, tag="a1o", name="a1o")
        a2o = qkv.tile([128, 72], f32, tag="a2o", name="a2o")

        wt = const.tile([128, 32, 128], bf16, tag="wt", name="wt")
        nc.sync.dma_start(out=wt, in_=wco_p.rearrange("t p co -> p t co"))

        def wco_v(ky, kx, k):
            return wt[:, ky * 8 + kx * 2 + k, :]

        wt8 = const.tile([128, 32, 128], f8, tag="wt8", name="wt8")
        nc.scalar.copy(out=wt8, in_=wt)

        def wco_pair(ky, kx):
            return wt8[:, ky * 8 + kx * 2 : ky * 8 + kx * 2 + 2, :]

        # x in ConvT layout (halo rows/cols zero) and the staged convT(x)+bias
        fpx = [
            [
                fpool.tile([128, 10, ROWW], bf16, tag=f"fpx{b}_{ch}", name=f"fpx{b}_{ch}")
                for ch in range(2)
            ]
            for b in range(B)
        ]
        stg = fpool.tile([128, 2, 2, B, 9, Wd], bf16, tag="stg", name="stg")
        for b in range(B):
            for ch in range(2):
                nc.gpsimd.memset(fpx[b][ch], 0.0)
                nc.scalar.copy(
                    out=fpx[b][ch][:, 1:9, 2:66],
                    in_=xt[:, b, ch, 2 : 2 + NL].rearrange("p (r w) -> p r w", w=Wd),
                )

        def bq_v(k):
            return fpk[:, k : k + 1]

        b1_v = fpk[0:CR, 2:3]
        b2q_v = fpk[0:CR, 3:4]
        b2k_v = fpk[CR : 2 * CR, 3:4]
        bco_v = fpk[:, 4:5]
        gm_v = fpk[0:1, 5:6]
        nege2_v = fpk[:, 6:7]  # -2.0 exp bias (fp8 range)

        # =================================================================
        # phases A (q path) + B (value) under the scoped const pack
        # =================================================================
        with (
            tc.tile_pool(name="cpA", bufs=1) as cpA,
            tc.tile_pool(name="qtmp", bufs=2) as qtmp,
            tc.tile_pool(name="qps", bufs=2, space="PSUM") as qps,
            tc.tile_pool(name="q2ps", bufs=1, space="PSUM") as q2ps,
            tc.tile_pool(name="q3ps", bufs=1, space="PSUM") as q3ps,
            tc.tile_pool(name="vps", bufs=1, space="PSUM") as vps,
            tc.tile_pool(name="vst", bufs=2) as vst,
        ):
            cp = cpA.tile([128, CPCOLS], bf16, tag="cp", name="cp")
            nc.sync.dma_start(out=cp, in_=cpack_p[:, :])

            def wq_v(k):
                return cp[:, OFF_WQ + k * 256 : OFF_WQ + (k + 1) * 256]

            def wv_v(k):
                return cp[:, OFF_WV + k * 256 : OFF_WV + (k + 1) * 256]

            def w1_v(t, k):
                o = OFF_W1 + (t * 2 + k) * CR
                return cp[:, o : o + CR]

            def w2_v(t):
                o = OFF_W2 + t * 64
                return cp[0:CR, o : o + 64]

            mask_v = cp[:, OFF_MASK : OFF_MASK + XW]
            bvb_v = cp[:, OFF_BVB : OFF_BVB + C]

            # x partial sums (for gamma*mean(x)) at s1p cols 128 + b*2 + k
            for b in range(B):
                for k in range(2):
                    cc = 128 + b * 2 + k
                    nc.vector.tensor_reduce(
                        out=s1p[:, cc : cc + 1],
                        in_=xt[:, b, k, 2 : 2 + NL],
                        axis=AX.X,
                        op=OP.add,
                    )

            # ---- phase A: q path per batch ------------------------------
            for b in range(B):
                q1_sb = []
                for mtile in range(2):
                    ps = qps.tile([128, XW], f32, tag="q1ps", name="q1ps")
                    for k in range(2):
                        for lo, hi in ((0, 512), (512, XW)):
                            nc.tensor.matmul(
                                ps[:, lo:hi],
                                wq_v(k)[:, mtile * 128 : (mtile + 1) * 128],
                                xt[:, b, k, lo:hi],
                                start=(k == 0),
                                stop=(k == 1),
                            )
                    q1 = qtmp.tile([128, XW], bf16, tag=f"q1_{mtile}", name=f"q1_{mtile}")
                    nc.scalar.activation(
                        out=q1, in_=ps, func=AF.Identity, bias=bq_v(mtile)
                    )
                    nc.vector.tensor_mul(q1, q1, mask_v)
                    q1_sb.append(q1)

                ps2 = q2ps.tile([CR, Q2W], f32, tag="q2ps", name="q2ps")
                for t in range(3):
                    for k in range(2):
                        st = t == 0 and k == 0
                        sp = t == 2 and k == 1
                        for lo, hi in ((0, 512), (512, Q2W)):
                            nc.tensor.matmul(
                                ps2[:, lo:hi],
                                w1_v(t, k),
                                q1_sb[k][:, lo + t : hi + t],
                                start=st,
                                stop=sp,
                            )
                q2 = qtmp.tile([CR, Q2W], bf16, tag="q2", name="q2")
                nc.scalar.activation(out=q2, in_=ps2, func=AF.Identity, bias=b1_v)
                nc.vector.tensor_mul(q2, q2, mask_v[:CR, 1 : 1 + Q2W])

                ps3 = q3ps.tile([2 * CR, NL], f32, tag="q3ps", name="q3ps")
                for t in range(3):
                    nc.tensor.matmul(
                        ps3,
                        w2_v(t),
                        q2[:, t : t + NL],
                        start=(t == 0),
                        stop=(t == 2),
                    )
                q3 = qtmp.tile([2 * CR, NL], f8, tag="q3", name="q3")
                nc.scalar.activation(
                    out=q3, in_=ps3, func=AF.Identity, bias=fpk[0 : 2 * CR, 3:4]
                )
                nc.sync.dma_start(
                    out=Q_all[CR * b : CR * (b + 1), :], in_=q3[0:CR, :]
                )
                nc.sync.dma_start(out=k_in[b], in_=q3[CR : 2 * CR, :])

            # single K AllGather for all 4 batches; high priority so the
            # scheduler keeps it AHEAD of the (bigger) V AllGather on the CC
            # queue -- phase C is gated on K
            with tc.high_priority(offset=1000):
                nc.gpsimd.collective_compute(
                    "AllGather",
                    OP.bypass,
                    replica_groups=RG,
                    ins=[k_in[:, :, :]],
                    outs=[k_out[:, :, :, :]],
                )

            # ---- phase B: value^T shards, fp8 ---------------------------
            for b in range(B):
                vstage = vst.tile([128, 4, C], f8, tag="vstage", name="vstage")
                for ms in range(4):
                    psv = vps.tile([128, C], f32, tag="vpsm", name="vpsm")
                    for k in range(2):
                        nc.tensor.matmul(
                            psv,
                            xt[:, b, k, 2 + ms * 128 : 2 + (ms + 1) * 128],
                            wv_v(k),
                            start=(k == 0),
                            stop=(k == 1),
                        )
                    nc.vector.tensor_add(vstage[:, ms, :], psv, bvb_v)
                nc.sync.dma_start(
                    out=v_in[b].rearrange("(ms p) c -> p ms c", p=128), in_=vstage
                )

            # assemble K_all from the gathered shards (per-b: the SBUF dst
            # must keep a single partition dim)
            for b in range(B):
                nc.sync.dma_start(
                    out=K_all[CR * b : CR * (b + 1), :].rearrange(
                        "c (g m) -> c g m", g=NCORES
                    ),
                    in_=k_out[:, b].rearrange("g c m -> c g m"),
                )

        # force the V AllGather BEHIND the K AllGather on the CC queue
        # (phase C is gated on K; the scheduler otherwise reorders them)
        with tc.tile_wait_until(0.06):
            nc.gpsimd.collective_compute(
                "AllGather",
                OP.bypass,
                replica_groups=RG,
                ins=[v_in[:, :, :]],
                outs=[v_out[:, :, :, :]],
            )

        # =================================================================
        # conv-x: ConvTranspose of the residual x, staged to SBUF (+bias).
        # Runs in the collective dead-zone; keeps the PE warm before C.
        # =================================================================
        NOUT = 9 * Wd  # 576 spatial outputs per (b, py, px)
        with tc.tile_pool(name="cvx", bufs=1, space="PSUM") as cvx:
            for py in range(2):
                for px in range(2):
                    psx = [
                        cvx.tile([128, NOUT], f32, tag=f"cvx{b}", name=f"cvx{b}")
                        for b in range(B)
                    ]
                    taps = [
                        (ky, kx, k)
                        for ky in (py, py + 2)
                        for kx in (px, px + 2)
                        for k in range(2)
                    ]
                    for ti, (ky, kx, k) in enumerate(taps):
                        ro = (py + ky) // 2 - py
                        ww = (px + kx) // 2 - 1
                        for b in range(B):
                            fp = fpx[b][k]
                            nc.tensor.matmul(
                                psx[b][:, 0:512],
                                wco_v(ky, kx, k),
                                fp[:, ro : ro + 8, 2 + ww : 66 + ww],
                                start=(ti == 0),
                                stop=(ti == len(taps) - 1),
                            )
                            nc.tensor.matmul(
                                psx[b][:, 512:NOUT],
                                wco_v(ky, kx, k),
                                fp[:, ro + 8, 2 + ww : 66 + ww],
                                start=(ti == 0),
                                stop=(ti == len(taps) - 1),
                            )
                    for b in range(B):
                        sv = stg[:, py, px, b]
                        pv = psx[b].rearrange("p (j w) -> p j w", w=Wd)
                        nc.scalar.activation(
                            out=sv[:, 1:9, :], in_=pv[:, 1:9, :],
                            func=AF.Identity, bias=bco_v,
                        )
                        nc.scalar.activation(
                            out=sv[:, 0:1, :], in_=pv[:, 0:1, :], func=AF.Copy,
                        )

        # =================================================================
        # phases C (QK + exp + denominators) and D (scale + fusion matmul)
        # =================================================================
        with tc.tile_pool(name="work", bufs=1) as work:
            # E in fp8e4 (exp bias -2 keeps E' <= ~130 < 448), stored as
            # m-tile PAIRS [128, 2, B, NL] for DoubleRow fusion matmuls
            e2 = [
                work.tile([128, 2, B, NL], f8, tag=f"e{t}", name=f"e{t}")
                for t in range(MT // 2)
            ]
            rb_sb = [
                work.tile([128, NL], bf16, tag=f"rb{mt}", name=f"rb{mt}")
                for mt in range(MT)
            ]

            with (
                tc.tile_pool(name="qk", bufs=2, space="PSUM") as qk,
                tc.tile_pool(name="sc", bufs=2) as sc,
            ):
                for mt in range(MT):
                    ps4 = qk.tile([128, B, NL], f32, tag="e4ps", name="e4ps")
                    for b in range(B):
                        nc.tensor.matmul(
                            ps4[:, b, :],
                            K_all[CR * b : CR * (b + 1), mt * 128 : (mt + 1) * 128],
                            Q_all[CR * b : CR * (b + 1), :],
                            start=True,
                            stop=True,
                            tile_position=(CR * b, 0),
                        )
                    ev = e2[mt // 2][:, mt % 2]  # [128, B, NL] view
                    # S1 partials: Scalar (per-b exp accum_out) for the
                    # tiles feeding the ARs (so AR inputs land with the exp,
                    # not at the DVE queue tail); DVE reduce for the middle
                    if mt < 16 or mt >= 24:
                        for b in range(B):
                            col = 4 * mt + b
                            nc.scalar.activation(
                                out=ev[:, b, :],
                                in_=ps4[:, b, :],
                                func=AF.Exp,
                                bias=nege2_v,
                                accum_out=s1p[:, col : col + 1],
                            )
                    else:
                        nc.scalar.activation(out=ev, in_=ps4, func=AF.Exp, bias=nege2_v)
                        nc.vector.tensor_reduce(
                            out=s1p[:, 4 * mt : 4 * mt + 4],
                            in_=ev,
                            axis=AX.X,
                            op=OP.add,
                        )
                    # S0 = sum_b E: add tree, mostly on GpSimd
                    s0f = sc.tile([128, NL], f32, tag="s0f", name="s0f")
                    if mt % 4 == 0:
                        t2 = sc.tile([128, 2, NL], bf16, tag="t2", name="t2")
                        nc.vector.tensor_add(t2, ev[:, 0:2, :], ev[:, 2:4, :])
                        nc.vector.tensor_add(s0f, t2[:, 0, :], t2[:, 1, :])
                    else:
                        s01 = sc.tile([128, NL], bf16, tag="s01", name="s01")
                        s23 = sc.tile([128, NL], bf16, tag="s23", name="s23")
                        nc.gpsimd.tensor_add(s01, ev[:, 0, :], ev[:, 1, :])
                        nc.gpsimd.tensor_add(s23, ev[:, 2, :], ev[:, 3, :])
                        nc.gpsimd.tensor_add(s0f, s01, s23)
                    rf = sc.tile([128, NL], f32, tag="rf", name="rf")
                    nc.vector.reciprocal_approx_fast(out=rf, in_=s0f)
                    if mt % 2 == 0:
                        nc.scalar.copy(out=rb_sb[mt], in_=rf)
                    else:
                        nc.vector.tensor_copy(rb_sb[mt], rf)

                    if mt == MT // 2 - 1:
                        nc.sync.dma_start(out=ar1_in[:, :], in_=s1p[:, 0:64])
                        nc.gpsimd.collective_compute(
                            "AllReduce", OP.add, replica_groups=RG,
                            ins=[ar1_in[:, :]], outs=[ar1_out[:, :]],
                        )
                        nc.sync.dma_start(out=a1o, in_=ar1_out[:, :])
                        r1f = sc.tile([128, 64], f32, tag="r1f", name="r1f")
                        nc.vector.reciprocal_approx_fast(out=r1f, in_=a1o)
                        nc.vector.tensor_copy(r1a, r1f)

                # second AR half: S1 cols 64..128 plus the x sums
                nc.sync.dma_start(out=ar2_in[:, 0:64], in_=s1p[:, 64:128])
                nc.sync.dma_start(out=ar2_in[:, 64:72], in_=s1p[:, 128:136])
                nc.gpsimd.collective_compute(
                    "AllReduce", OP.add, replica_groups=RG,
                    ins=[ar2_in[:, :]], outs=[ar2_out[:, :]],
                )
                nc.sync.dma_start(out=a2o, in_=ar2_out[:, :])
                r2f = sc.tile([128, 64], f32, tag="r2f", name="r2f")
                nc.vector.reciprocal_approx_fast(out=r2f, in_=a2o[:, 0:64])
                nc.vector.tensor_copy(r1b, r2f)

                # g_bcast[p, b] = gamma * mean(x[b])
                xps = sc.tile([1, 8], f32, tag="xps", name="xps")
                nc.gpsimd.tensor_reduce(
                    out=xps, in_=a2o[:, 64:72], axis=AX.C, op=OP.add
                )
                xv = xps.rearrange("p (b k) -> p b k", b=B)
                g0 = sc.tile([1, B], f32, tag="g0", name="g0")
                nc.vector.tensor_add(g0, xv[:, :, 0], xv[:, :, 1])
                nc.vector.tensor_scalar(
                    out=g0,
                    in0=g0,
                    scalar1=gm_v,
                    scalar2=float(4.0 / (C * HW)),
                    op0=OP.mult,
                    op1=OP.mult,
                )
                nc.sync.dma_start(out=g_dram[:, :], in_=g0)
                nc.sync.dma_start(
                    out=g_bcast,
                    in_=bass.AP(
                        tensor=g_dram.tensor,
                        offset=g_dram.offset,
                        ap=[[0, 128], [1, B]],
                    ),
                )

            # raw fusion in fp8 (|fusion| ~ 13 << 448); gamma*mean(x) is
            # applied in the phase-E epilogue (convT is linear, g is a
            # per-batch scalar)
            ff8 = [
                work.tile([128, 2, 10, ROWW], f8, tag=f"ff8{b}", name=f"ff8{b}")
                for b in range(B)
            ]
            for b in range(B):
                nc.gpsimd.memset(ff8[b], 0.0)

            # ---- phase D: A = E*(1/S0 + 1/S1) in place; fusion matmuls --
            with (
                tc.tile_pool(name="fus", bufs=1, space="PSUM") as fus,
                tc.tile_pool(name="vtp", bufs=4) as vtp,
            ):
                fusion_ps = [
                    [
                        fus.tile([128, NL], f32, tag=f"f{b}_{ch}", name=f"f{b}_{ch}")
                        for ch in range(2)
                    ]
                    for b in range(B)
                ]
                NP = MT // 2
                for t in range(NP):
                    g = t // 2
                    ml = (t % 2) * 256
                    vt8 = vtp.tile([128, 2, B, C], f8, tag="vt8", name="vt8")
                    for b in range(B):
                        nc.sync.dma_start(
                            out=vt8[:, :, b, :],
                            in_=v_out[g, b, ml : ml + 256, :].rearrange(
                                "(two p) c -> p two c", p=128
                            ),
                        )
                    et = e2[t]
                    for par in range(2):
                        mt = 2 * t + par
                        r1h = r1a if mt < 16 else r1b
                        cb = (4 * mt) % 64
                        for b in range(B):
                            nc.vector.scalar_tensor_tensor(
                                out=et[:, par, b, :],
                                in0=rb_sb[mt],
                                scalar=r1h[:, cb + b : cb + b + 1],
                                in1=et[:, par, b, :],
                                op0=OP.add,
                                op1=OP.mult,
                            )
                    for b in range(B):
                        for ch in range(2):
                            nc.tensor.matmul(
                                fusion_ps[b][ch],
                                vt8[:, :, b, ch * 128 : (ch + 1) * 128],
                                et[:, :, b, :],
                                start=(t == 0),
                                stop=(t == NP - 1),
                                perf_mode=mybir.MatmulPerfMode.DoubleRow,
                            )

                # ---- stage raw fusion to fp8 conv layout ----------------
                for b in range(B):
                    for ch in range(2):
                        # scale by 1/4: TRN fp8e4 max-normal is 240 and
                        # |fusion| reaches ~275; the epilogue g absorbs the 4x
                        nc.scalar.activation(
                            out=ff8[b][:, ch, 1:9, 2:66],
                            in_=fusion_ps[b][ch].rearrange("p (r w) -> p r w", w=Wd),
                            func=AF.Copy,
                            scale=0.25,
                        )

        # =================================================================
        # phase E: ConvTranspose2d of the fusion branch (fp8 DoubleRow over
        # the two c-chunks), epilogue out = g_b * conv_f + staged conv_x
        # =================================================================
        with (
            tc.tile_pool(name="ostp", bufs=2) as ostp,
            tc.tile_pool(name="cps", bufs=1, space="PSUM") as cps,
        ):
            for py in range(2):
                ost = ostp.tile([128, B, 9, 2 * Wd], bf16, tag="ost", name="ost")
                for px in range(2):
                    pss = [
                        cps.tile([128, NOUT], f32, tag=f"cps{b}", name=f"cps{b}")
                        for b in range(B)
                    ]
                    taps = [
                        (ky, kx)
                        for ky in (py, py + 2)
                        for kx in (px, px + 2)
                    ]
                    for ti, (ky, kx) in enumerate(taps):
                        ro = (py + ky) // 2 - py
                        ww = (px + kx) // 2 - 1
                        for b in range(B):
                            fp = ff8[b]
                            nc.tensor.matmul(
                                pss[b][:, 0:512],
                                wco_pair(ky, kx),
                                fp[:, :, ro : ro + 8, 2 + ww : 66 + ww],
                                start=(ti == 0),
                                stop=(ti == len(taps) - 1),
                                perf_mode=mybir.MatmulPerfMode.DoubleRow,
                            )
                            nc.tensor.matmul(
                                pss[b][:, 512:NOUT],
                                wco_pair(ky, kx),
                                fp[:, :, ro + 8, 2 + ww : 66 + ww],
                                start=(ti == 0),
                                stop=(ti == len(taps) - 1),
                                perf_mode=mybir.MatmulPerfMode.DoubleRow,
                            )
                    for b in range(B):
                        ov = ost[:, b].rearrange("p j (w q) -> p j w q", q=2)[
                            :, :, :, px
                        ]
                        psv = pss[b].rearrange("p (j w) -> p j w", w=Wd)
                        nc.vector.scalar_tensor_tensor(
                            out=ov,
                            in0=psv,
                            scalar=g_bcast[:, b : b + 1],
                            in1=stg[:, py, px, b],
                            op0=OP.mult,
                            op1=OP.add,
                        )
                for b in range(B):
                    nc.sync.dma_start(
                        out=out_p[b].rearrange("c (j t) w -> c j t w", t=2)[
                            :, :, 1 - py, :
                        ],
                        in_=ost[:, b],
                    )

    nc.finalize()
    return nc


# ---------------------------------------------------------------------------
# host side
# ---------------------------------------------------------------------------
def _host_prep(x, wq, bq, wv, bv, w_adj1, b_adj1, w_adj2, b_adj2, gamma, w_co, b_co):
    import ml_dtypes

    bf16 = ml_dtypes.bfloat16
    x = np.asarray(x, np.float32).reshape(B, C, HW)
    xpad = np.zeros((B, C, HW + 4), np.float32)
    xpad[:, :, 2 : 2 + HW] = x

    wqT = np.ascontiguousarray(np.asarray(wq, np.float32).T)  # [C, C]
    wvT = np.ascontiguousarray(np.asarray(wv, np.float32).T)

    # grouped conv -> block-diagonal [3, 256, 32]
    w1 = np.zeros((3, C, CR), np.float32)
    wa1 = np.asarray(w_adj1, np.float32)  # [32, 8, 3]
    for g in range(CR):
        w1[:, g * 8 : (g + 1) * 8, g] = wa1[g].T  # [8,3] -> [3,8]

    # conv2 with output channels permuted to [query(32) | key(32)]
    wa2 = np.asarray(w_adj2, np.float32)  # [64, 32, 3]
    perm = np.concatenate([np.arange(0, 64, 2), np.arange(1, 64, 2)])
    w2 = np.ascontiguousarray(wa2[perm].transpose(2, 1, 0))  # [3, 32, 64]
    b2p = np.asarray(b_adj2, np.float32)[perm]

    # convT weights: flip, swap I/O -> [ky, kx, c_in, c_out] -> [32,128,128]
    wt = np.flip(np.asarray(w_co, np.float32), (2, 3)).transpose(1, 0, 2, 3)
    wco = np.ascontiguousarray(
        wt.transpose(2, 3, 1, 0).reshape(4, 4, 2, 128, 128).reshape(32, 128, 128)
    ).astype(bf16)

    # const pack (mask differs per core; rest shared)
    cbase = np.zeros((128, CPCOLS), np.float32)
    for k in range(2):
        cbase[:, OFF_WQ + k * 256 : OFF_WQ + (k + 1) * 256] = wqT[
            k * 128 : (k + 1) * 128, :
        ]
        cbase[:, OFF_WV + k * 256 : OFF_WV + (k + 1) * 256] = wvT[
            k * 128 : (k + 1) * 128, :
        ]
    for t in range(3):
        for k in range(2):
            o = OFF_W1 + (t * 2 + k) * CR
            cbase[:, o : o + CR] = w1[t, k * 128 : (k + 1) * 128, :]
        cbase[0:CR, OFF_W2 + t * 64 : OFF_W2 + (t + 1) * 64] = w2[t]
    cbase[:, OFF_BVB : OFF_BVB + C] = np.asarray(bv, np.float32)[None, :]

    # f32 pack: bq k0/k1, b1, b2(perm), bco, gamma
    fpack = np.zeros((128, 8), np.float32)
    bqf = np.asarray(bq, np.float32)
    fpack[:, 0] = bqf[0:128]
    fpack[:, 1] = bqf[128:256]
    fpack[0:CR, 2] = np.asarray(b_adj1, np.float32)
    fpack[0 : 2 * CR, 3] = b2p
    fpack[:, 4] = np.asarray(b_co, np.float32)
    fpack[0, 5] = np.asarray(gamma, np.float32).reshape(-1)[0]
    fpack[:, 6] = -2.0
    fpack = np.ascontiguousarray(fpack)

    in_maps = []
    for i in range(NCORES):
        n0 = i * NL
        xsl = xpad[:, :, n0 : n0 + XW]  # [B, C, XW]
        xpk = np.ascontiguousarray(
            xsl.reshape(B, 2, 128, XW).transpose(2, 0, 1, 3).astype(bf16)
        )
        j = np.arange(XW)
        valid = ((n0 - 2 + j) >= 0) & ((n0 - 2 + j) < HW)
        cpk = cbase.copy()
        cpk[:, OFF_MASK : OFF_MASK + XW] = valid.astype(np.float32)[None, :]
        in_maps.append(
            dict(
                cpack=np.ascontiguousarray(cpk.astype(bf16)),
                fpack=fpack,
                xpack=xpk,
                wco=wco,
            )
        )
    return in_maps


def _stitch(outs):
    full = np.zeros((B, C // 2, 2 * H, 2 * Wd), np.float32)
    for i in range(NCORES):
        y0 = 16 * i - 1
        lo = max(0, y0)
        hi = min(2 * H, y0 + OUTROWS)
        full[:, :, lo:hi, :] += np.asarray(
            outs[i][:, :, lo - y0 : hi - y0, :], np.float32
        )
    return full


def _get_nc():
    if "nc" not in _CACHE:
        _CACHE["nc"] = build_module()
    return _CACHE["nc"]


def run_spmd(in_maps, trace=False, **kw):
    from concourse.bass_utils import run_bass_kernel_spmd

    nc = _get_nc()
    return run_bass_kernel_spmd(
        nc, in_maps, core_ids=list(range(NCORES)), trace=trace, **kw
    )


def kernel(x, wq, bq, wv, bv, w_adj1, b_adj1, w_adj2, b_adj2, gamma, w_co, b_co):
    in_maps = _host_prep(
        x, wq, bq, wv, bv, w_adj1, b_adj1, w_adj2, b_adj2, gamma, w_co, b_co
    )
    res = run_spmd(in_maps)
    full = _stitch([r["out"] for r in res.results])
    # slab rows 0,1 carry no bias (the neighbor's rows complete them);
    # global row 0 has no neighbor, so add the bias here.
    full[:, :, 0, :] += np.asarray(b_co, np.float32)[None, :, None]
    return full.astype(np.float32)


# revision 32
# speedup vs baseline: 1.0016x; 1.0016x over previous
"""BidirectionalAttention Trainium2 Bass kernel — 8-core SPMD, v2.

Decomposition (same math as the verified baseline):
  q path : 1x1 conv (matmul) -> grouped conv1d k=3 -> conv1d k=3
  attn   : E = exp(q^T k); both softmaxes share one exp:
             attn_f + attn_b = E * (1/S0[n,m] + 1/S1[b,m])
             S0 = sum_b E  (batch softmax denom, axis=0)
             S1 = sum_n E  (row softmax denom, axis=1) -> two AllReduces
  fusion = value @ (attn_f+attn_b)^T scaled by gamma*mean(x_b), + x
  ConvTranspose2d(k=4,s=2,p=1) via the 4-subkernel parity decomposition,
  18-row output slabs with additive 2-row seams stitched on the host.

v2 performance changes vs the baseline:
  - K/Q/V in fp8e4 (K and V AllGathers halve; the attention branch output
    is scaled by gamma*mean(x) ~ 1e-3 so it tolerates fp8 easily).
    V is upcast to bf16 on the Scalar engine before the fusion matmuls.
  - E stays bf16 (DVE 2x perf mode requires 2-byte dtypes end-to-end).
  - One K AllGather for all 4 batches (was 4, each paying the ~15us ncfw
    floor).  CC-queue order: K-AG -> V-AG -> AR1 -> AR2, sized so each
    hides under local compute.
  - Phase C: one exp per m-tile, S1 via a single DVE tensor_reduce into a
    bf16 row (2x mode), S0 via an add tree split DVE/GpSimd by mt parity,
    1/S0 cached in bf16 for phase D (32 x 1KB/lane).
  - Phase D: A = (1/S0 + 1/S1) * E as ONE scalar_tensor_tensor per batch
    (replaces 4 tensor_scalars + a [128,2048] multiply), all operands bf16
    so DVE runs 2x.  Fusion matmuls then stream back-to-back to keep the
    PE warm (HAM throttling halved the baseline's matmul rate).
  - Coalesced DMAs: one const pack, one x pack, per-b V stages, one wco
    load, one output DMA per parity row.  Output returned in bf16.
"""

import numpy as np

B = 4
C = 256
H = 64
Wd = 64
HW = H * Wd            # 4096
CR = 32                # C // 8
NCORES = 8
NL = HW // NCORES      # 512 owned attention rows (n) per core
HL = H // NCORES       # 8 owned image rows per core
MT = HW // 128         # 32 m-tiles of 128
XW = NL + 4            # x slab width (n halo +-2 for the two k=3 convs)
Q2W = NL + 2           # q2 width (halo +-1 for conv2)
ROWW = 68              # fusion_pad row width: [0,1]=zero, 2..65 data, [66,67]=zero
OUTROWS = 2 * HL + 2   # 18 output rows per core (2-row overlaps, host-stitched)

# const-pack column offsets (bf16 elements)
OFF_WQ = 0             # [2, 256]
OFF_WV = 512           # [2, 256]
OFF_W1 = 1024          # [3, 2, 32]
OFF_MASK = 1216        # [516]
OFF_BVB = 1732         # [256]
OFF_W2 = 1988          # rows 0:32, [3, 64]
CPCOLS = 2180

_CACHE = {}


# ---------------------------------------------------------------------------
# device module
# ---------------------------------------------------------------------------
def build_module():
    from contextlib import ExitStack

    import concourse.bass as bass
    import concourse.mybir as mybir
    from concourse import bacc
    from concourse.tile import TileContext

    f32 = mybir.dt.float32
    bf16 = mybir.dt.bfloat16
    f8 = mybir.dt.float8e4
    AF = mybir.ActivationFunctionType
    OP = mybir.AluOpType
    AX = mybir.AxisListType

    nc = bacc.Bacc(num_devices=NCORES)
    RG = [list(range(NCORES))]

    # ---- parameters (per-core) -------------------------------------------
    cpack_p = nc.declare_dram_parameter("cpack", [128, CPCOLS], bf16, isOutput=False)
    fpack_p = nc.declare_dram_parameter("fpack", [128, 8], f32, isOutput=False)
    xpack_p = nc.declare_dram_parameter("xpack", [128, B, 2, XW], bf16, isOutput=False)
    wco_p = nc.declare_dram_parameter("wco", [32, 128, 128], bf16, isOutput=False)
    out_p = nc.declare_dram_parameter(
        "out", [B, C // 2, OUTROWS, 2 * Wd], bf16, isOutput=True
    )

    with TileContext(nc) as tc, ExitStack() as ctx:
        # ---- long-lived pools -------------------------------------------
        const = ctx.enter_context(tc.tile_pool(name="const", bufs=1))
        xpool = ctx.enter_context(tc.tile_pool(name="xp", bufs=1))
        qkv = ctx.enter_context(tc.tile_pool(name="qkv", bufs=1))
        fpool = ctx.enter_context(tc.tile_pool(name="fp", bufs=1))
        dram = ctx.enter_context(tc.tile_pool(name="dram", bufs=1, space="DRAM"))

        # ---- DRAM bounce buffers ----------------------------------------
        k_in = dram.tile([B, CR, NL], f8, tag="k_in", name="k_in")
        k_out = dram.tile(
            [NCORES, B, CR, NL], f8, tag="k_out", name="k_out"
        )
        v_in = dram.tile([B, NL, C], f8, tag="v_in", name="v_in")
        v_out = dram.tile(
            [NCORES, B, NL, C], f8, tag="v_out", name="v_out"
        )
        ar1_in = dram.tile([128, 64], f32, tag="ar1_in", name="ar1_in")
        ar1_out = dram.tile(
            [128, 64], f32, tag="ar1_out", name="ar1_out"
        )
        ar2_in = dram.tile([128, 72], f32, tag="ar2_in", name="ar2_in")
        ar2_out = dram.tile(
            [128, 72], f32, tag="ar2_out", name="ar2_out"
        )
        g_dram = dram.tile([1, B], f32, tag="g_dram", name="g_dram")
        warm_in = dram.tile([1, 4], f32, tag="warm_in", name="warm_in")
        warm_out = dram.tile([NCORES, 4], f32, tag="warm_out", name="warm_out")

        # warm-up rendezvous: absorb the cross-core NEFF-start skew under
        # phase A instead of paying it at the first real collective
        with tc.high_priority():
            nc.gpsimd.collective_compute(
                "AllGather", OP.bypass, replica_groups=RG,
                ins=[warm_in[:, :]], outs=[warm_out[:, :]],
            )

        # ---- persistent SBUF state --------------------------------------
        fpk = const.tile([128, 8], f32, tag="fpk", name="fpk")
        nc.sync.dma_start(out=fpk, in_=fpack_p[:, :])
        xt = xpool.tile([128, B, 2, XW], bf16, tag="xt", name="xt")
        nc.sync.dma_start(out=xt, in_=xpack_p[:, :, :, :])

        s1p = qkv.tile([128, 136], f32, tag="s1p", name="s1p")
        Q_all = qkv.tile([128, NL], f8, tag="Q", name="Q")
        K_all = qkv.tile([128, HW], f8, tag="K", name="K")
        r1a = qkv.tile([128, 64], bf16, tag="r1a", name="r1a")  # 1/S1, mt<16
        r1b = qkv.tile([128, 64], bf16, tag="r1b", name="r1b")  # 1/S1, mt>=16
        g_bcast = qkv.tile([128, B], f32, tag="gbc", name="gbc")
        a1o = qkv.tile([128, 64], f32, tag="a1o", name="a1o")
        a2o = qkv.tile([128, 72], f32, tag="a2o", name="a2o")

        wt = const.tile([128, 32, 128], bf16, tag="wt", name="wt")
        nc.sync.dma_start(out=wt, in_=wco_p.rearrange("t p co -> p t co"))

        def wco_v(ky, kx, k):
            return wt[:, ky * 8 + kx * 2 + k, :]

        wt8 = const.tile([128, 32, 128], f8, tag="wt8", name="wt8")
        nc.scalar.copy(out=wt8, in_=wt)

        def wco_pair(ky, kx):
            return wt8[:, ky * 8 + kx * 2 : ky * 8 + kx * 2 + 2, :]

        # x in ConvT layout (halo rows/cols zero) and the staged convT(x)+bias
        fpx = [
            [
                fpool.tile([128, 10, ROWW], bf16, tag=f"fpx{b}_{ch}", name=f"fpx{b}_{ch}")
                for ch in range(2)
            ]
            for b in range(B)
        ]
        stg = fpool.tile([128, 2, 2, B, 9, Wd], bf16, tag="stg", name="stg")
        for b in range(B):
            for ch in range(2):
                nc.gpsimd.memset(fpx[b][ch], 0.0)
                nc.scalar.copy(
                    out=fpx[b][ch][:, 1:9, 2:66],
                    in_=xt[:, b, ch, 2 : 2 + NL].rearrange("p (r w) -> p r w", w=Wd),
                )

        def bq_v(k):
            return fpk[:, k : k + 1]

        b1_v = fpk[0:CR, 2:3]
        b2q_v = fpk[0:CR, 3:4]
        b2k_v = fpk[CR : 2 * CR, 3:4]
        bco_v = fpk[:, 4:5]
        gm_v = fpk[0:1, 5:6]
        nege2_v = fpk[:, 6:7]  # -2.0 exp bias (fp8 range)

        # =================================================================
        # phases A (q path) + B (value) under the scoped const pack
        # =================================================================
        with (
            tc.tile_pool(name="cpA", bufs=1) as cpA,
            tc.tile_pool(name="qtmp", bufs=2) as qtmp,
            tc.tile_pool(name="qps", bufs=2, space="PSUM") as qps,
            tc.tile_pool(name="q2ps", bufs=1, space="PSUM") as q2ps,
            tc.tile_pool(name="q3ps", bufs=1, space="PSUM") as q3ps,
            tc.tile_pool(name="vps", bufs=1, space="PSUM") as vps,
            tc.tile_pool(name="vst", bufs=2) as vst,
        ):
            cp = cpA.tile([128, CPCOLS], bf16, tag="cp", name="cp")
            nc.sync.dma_start(out=cp, in_=cpack_p[:, :])

            def wq_v(k):
                return cp[:, OFF_WQ + k * 256 : OFF_WQ + (k + 1) * 256]

            def wv_v(k):
                return cp[:, OFF_WV + k * 256 : OFF_WV + (k + 1) * 256]

            def w1_v(t, k):
                o = OFF_W1 + (t * 2 + k) * CR
                return cp[:, o : o + CR]

            def w2_v(t):
                o = OFF_W2 + t * 64
                return cp[0:CR, o : o + 64]

            mask_v = cp[:, OFF_MASK : OFF_MASK + XW]
            bvb_v = cp[:, OFF_BVB : OFF_BVB + C]

            # x partial sums (for gamma*mean(x)) at s1p cols 128 + b*2 + k
            for b in range(B):
                for k in range(2):
                    cc = 128 + b * 2 + k
                    nc.vector.tensor_reduce(
                        out=s1p[:, cc : cc + 1],
                        in_=xt[:, b, k, 2 : 2 + NL],
                        axis=AX.X,
                        op=OP.add,
                    )

            # ---- phase A: q path per batch ------------------------------
            for b in range(B):
                q1_sb = []
                for mtile in range(2):
                    ps = qps.tile([128, XW], f32, tag="q1ps", name="q1ps")
                    for k in range(2):
                        for lo, hi in ((0, 512), (512, XW)):
                            nc.tensor.matmul(
                                ps[:, lo:hi],
                                wq_v(k)[:, mtile * 128 : (mtile + 1) * 128],
                                xt[:, b, k, lo:hi],
                                start=(k == 0),
                                stop=(k == 1),
                            )
                    q1 = qtmp.tile([128, XW], bf16, tag=f"q1_{mtile}", name=f"q1_{mtile}")
                    nc.scalar.activation(
                        out=q1, in_=ps, func=AF.Identity, bias=bq_v(mtile)
                    )
                    nc.vector.tensor_mul(q1, q1, mask_v)
                    q1_sb.append(q1)

                ps2 = q2ps.tile([CR, Q2W], f32, tag="q2ps", name="q2ps")
                for t in range(3):
                    for k in range(2):
                        st = t == 0 and k == 0
                        sp = t == 2 and k == 1
                        for lo, hi in ((0, 512), (512, Q2W)):
                            nc.tensor.matmul(
                                ps2[:, lo:hi],
                                w1_v(t, k),
                                q1_sb[k][:, lo + t : hi + t],
                                start=st,
                                stop=sp,
                            )
                q2 = qtmp.tile([CR, Q2W], bf16, tag="q2", name="q2")
                nc.scalar.activation(out=q2, in_=ps2, func=AF.Identity, bias=b1_v)
                nc.vector.tensor_mul(q2, q2, mask_v[:CR, 1 : 1 + Q2W])

                ps3 = q3ps.tile([2 * CR, NL], f32, tag="q3ps", name="q3ps")
                for t in range(3):
                    nc.tensor.matmul(
                        ps3,
                        w2_v(t),
                        q2[:, t : t + NL],
                        start=(t == 0),
                        stop=(t == 2),
                    )
                q3 = qtmp.tile([2 * CR, NL], f8, tag="q3", name="q3")
                nc.scalar.activation(
                    out=q3, in_=ps3, func=AF.Identity, bias=fpk[0 : 2 * CR, 3:4]
                )
                nc.sync.dma_start(
                    out=Q_all[CR * b : CR * (b + 1), :], in_=q3[0:CR, :]
                )
                nc.sync.dma_start(out=k_in[b], in_=q3[CR : 2 * CR, :])

            # single K AllGather for all 4 batches; high priority so the
            # scheduler keeps it AHEAD of the (bigger) V AllGather on the CC
            # queue -- phase C is gated on K
            with tc.high_priority(offset=1000):
                nc.gpsimd.collective_compute(
                    "AllGather",
                    OP.bypass,
                    replica_groups=RG,
                    ins=[k_in[:, :, :]],
                    outs=[k_out[:, :, :, :]],
                )

            # ---- phase B: value^T shards, fp8 ---------------------------
            for b in range(B):
                vstage = vst.tile([128, 4, C], f8, tag="vstage", name="vstage")
                for ms in range(4):
                    psv = vps.tile([128, C], f32, tag="vpsm", name="vpsm")
                    for k in range(2):
                        nc.tensor.matmul(
                            psv,
                            xt[:, b, k, 2 + ms * 128 : 2 + (ms + 1) * 128],
                            wv_v(k),
                            start=(k == 0),
                            stop=(k == 1),
                        )
                    nc.vector.tensor_add(vstage[:, ms, :], psv, bvb_v)
                nc.sync.dma_start(
                    out=v_in[b].rearrange("(ms p) c -> p ms c", p=128), in_=vstage
                )

            # assemble K_all from the gathered shards (per-b: the SBUF dst
            # must keep a single partition dim)
            for b in range(B):
                nc.sync.dma_start(
                    out=K_all[CR * b : CR * (b + 1), :].rearrange(
                        "c (g m) -> c g m", g=NCORES
                    ),
                    in_=k_out[:, b].rearrange("g c m -> c g m"),
                )

        # force the V AllGather BEHIND the K AllGather on the CC queue
        # (phase C is gated on K; the scheduler otherwise reorders them)
        with tc.tile_wait_until(0.06):
            nc.gpsimd.collective_compute(
                "AllGather",
                OP.bypass,
                replica_groups=RG,
                ins=[v_in[:, :, :]],
                outs=[v_out[:, :, :, :]],
            )

        # =================================================================
        # conv-x: ConvTranspose of the residual x, staged to SBUF (+bias).
        # Runs in the collective dead-zone; keeps the PE warm before C.
        # =================================================================
        NOUT = 9 * Wd  # 576 spatial outputs per (b, py, px)
        with tc.tile_pool(name="cvx", bufs=1, space="PSUM") as cvx:
            for py in range(2):
                for px in range(2):
                    psx = [
                        cvx.tile([128, NOUT], f32, tag=f"cvx{b}", name=f"cvx{b}")
                        for b in range(B)
                    ]
                    taps = [
                        (ky, kx, k)
                        for ky in (py, py + 2)
                        for kx in (px, px + 2)
                        for k in range(2)
                    ]
                    for ti, (ky, kx, k) in enumerate(taps):
                        ro = (py + ky) // 2 - py
                        ww = (px + kx) // 2 - 1
                        for b in range(B):
                            fp = fpx[b][k]
                            nc.tensor.matmul(
                                psx[b][:, 0:512],
                                wco_v(ky, kx, k),
                                fp[:, ro : ro + 8, 2 + ww : 66 + ww],
                                start=(ti == 0),
                                stop=(ti == len(taps) - 1),
                            )
                            nc.tensor.matmul(
                                psx[b][:, 512:NOUT],
                                wco_v(ky, kx, k),
                                fp[:, ro + 8, 2 + ww : 66 + ww],
                                start=(ti == 0),
                                stop=(ti == len(taps) - 1),
                            )
                    for b in range(B):
                        sv = stg[:, py, px, b]
                        pv = psx[b].rearrange("p (j w) -> p j w", w=Wd)
                        nc.scalar.activation(
                            out=sv[:, 1:9, :], in_=pv[:, 1:9, :],
                            func=AF.Identity, bias=bco_v,
                        )
                        nc.scalar.activation(
                            out=sv[:, 0:1, :], in_=pv[:, 0:1, :], func=AF.Copy,
                        )

        # =================================================================
        # phases C (QK + exp + denominators) and D (scale + fusion matmul)
        # =================================================================
        with tc.tile_pool(name="work", bufs=1) as work:
            # E in fp8e4 (exp bias -2 keeps E' <= ~130 < 448), stored as
            # m-tile PAIRS [128, 2, B, NL] for DoubleRow fusion matmuls
            e2 = [
                work.tile([128, 2, B, NL], f8, tag=f"e{t}", name=f"e{t}")
                for t in range(MT // 2)
            ]
            rb_sb = [
                work.tile([128, NL], bf16, tag=f"rb{mt}", name=f"rb{mt}")
                for mt in range(MT)
            ]

            with (
                tc.tile_pool(name="qk", bufs=2, space="PSUM") as qk,
                tc.tile_pool(name="sc", bufs=2) as sc,
            ):
                for mt in range(MT):
                    ps4 = qk.tile([128, B, NL], f32, tag="e4ps", name="e4ps")
                    for b in range(B):
                        nc.tensor.matmul(
                            ps4[:, b, :],
                            K_all[CR * b : CR * (b + 1), mt * 128 : (mt + 1) * 128],
                            Q_all[CR * b : CR * (b + 1), :],
                            start=True,
                            stop=True,
                            tile_position=(CR * b, 0),
                        )
                    ev = e2[mt // 2][:, mt % 2]  # [128, B, NL] view
                    # S1 partials: Scalar (per-b exp accum_out) for the
                    # tiles feeding the ARs (so AR inputs land with the exp,
                    # not at the DVE queue tail); DVE reduce for the middle
                    if mt % 8 < 3 or mt >= 29:
                        for b in range(B):
                            col = 4 * mt + b
                            nc.scalar.activation(
                                out=ev[:, b, :],
                                in_=ps4[:, b, :],
                                func=AF.Exp,
                                bias=nege2_v,
                                accum_out=s1p[:, col : col + 1],
                            )
                    else:
                        nc.scalar.activation(out=ev, in_=ps4, func=AF.Exp, bias=nege2_v)
                        nc.vector.tensor_reduce(
                            out=s1p[:, 4 * mt : 4 * mt + 4],
                            in_=ev,
                            axis=AX.X,
                            op=OP.add,
                        )
                    # S0 = sum_b E: add tree, mostly on GpSimd
                    s0f = sc.tile([128, NL], f32, tag="s0f", name="s0f")
                    if mt % 4 == 0:
                        t2 = sc.tile([128, 2, NL], bf16, tag="t2", name="t2")
                        nc.vector.tensor_add(t2, ev[:, 0:2, :], ev[:, 2:4, :])
                        nc.vector.tensor_add(s0f, t2[:, 0, :], t2[:, 1, :])
                    else:
                        s01 = sc.tile([128, NL], bf16, tag="s01", name="s01")
                        s23 = sc.tile([128, NL], bf16, tag="s23", name="s23")
                        nc.gpsimd.tensor_add(s01, ev[:, 0, :], ev[:, 1, :])
                        nc.gpsimd.tensor_add(s23, ev[:, 2, :], ev[:, 3, :])
                        nc.gpsimd.tensor_add(s0f, s01, s23)
                    rf = sc.tile([128, NL], f32, tag="rf", name="rf")
                    nc.vector.reciprocal_approx_fast(out=rf, in_=s0f)
                    if mt % 2 == 0:
                        nc.scalar.copy(out=rb_sb[mt], in_=rf)
                    else:
                        nc.vector.tensor_copy(rb_sb[mt], rf)

                    if mt == MT // 2 - 1:
                        nc.sync.dma_start(out=ar1_in[:, :], in_=s1p[:, 0:64])
                        nc.gpsimd.collective_compute(
                            "AllReduce", OP.add, replica_groups=RG,
                            ins=[ar1_in[:, :]], outs=[ar1_out[:, :]],
                        )
                        nc.sync.dma_start(out=a1o, in_=ar1_out[:, :])
                        r1f = sc.tile([128, 64], f32, tag="r1f", name="r1f")
                        nc.vector.reciprocal_approx_fast(out=r1f, in_=a1o)
                        nc.vector.tensor_copy(r1a, r1f)

                # second AR half: S1 cols 64..128 plus the x sums
                nc.sync.dma_start(out=ar2_in[:, 0:64], in_=s1p[:, 64:128])
                nc.sync.dma_start(out=ar2_in[:, 64:72], in_=s1p[:, 128:136])
                nc.gpsimd.collective_compute(
                    "AllReduce", OP.add, replica_groups=RG,
                    ins=[ar2_in[:, :]], outs=[ar2_out[:, :]],
                )
                nc.sync.dma_start(out=a2o, in_=ar2_out[:, :])
                r2f = sc.tile([128, 64], f32, tag="r2f", name="r2f")
                nc.vector.reciprocal_approx_fast(out=r2f, in_=a2o[:, 0:64])
                nc.vector.tensor_copy(r1b, r2f)

                # g_bcast[p, b] = gamma * mean(x[b])
                xps = sc.tile([1, 8], f32, tag="xps", name="xps")
                nc.gpsimd.tensor_reduce(
                    out=xps, in_=a2o[:, 64:72], axis=AX.C, op=OP.add
                )
                xv = xps.rearrange("p (b k) -> p b k", b=B)
                g0 = sc.tile([1, B], f32, tag="g0", name="g0")
                nc.vector.tensor_add(g0, xv[:, :, 0], xv[:, :, 1])
                nc.vector.tensor_scalar(
                    out=g0,
                    in0=g0,
                    scalar1=gm_v,
                    scalar2=float(4.0 / (C * HW)),
                    op0=OP.mult,
                    op1=OP.mult,
                )
                nc.sync.dma_start(out=g_dram[:, :], in_=g0)
                nc.sync.dma_start(
                    out=g_bcast,
                    in_=bass.AP(
                        tensor=g_dram.tensor,
                        offset=g_dram.offset,
                        ap=[[0, 128], [1, B]],
                    ),
                )

            # raw fusion in fp8 (|fusion| ~ 13 << 448); gamma*mean(x) is
            # applied in the phase-E epilogue (convT is linear, g is a
            # per-batch scalar)
            ff8 = [
                work.tile([128, 2, 10, ROWW], f8, tag=f"ff8{b}", name=f"ff8{b}")
                for b in range(B)
            ]
            for b in range(B):
                nc.gpsimd.memset(ff8[b], 0.0)

            # ---- phase D: A = E*(1/S0 + 1/S1) in place; fusion matmuls --
            with (
                tc.tile_pool(name="fus", bufs=1, space="PSUM") as fus,
                tc.tile_pool(name="vtp", bufs=4) as vtp,
            ):
                fusion_ps = [
                    [
                        fus.tile([128, NL], f32, tag=f"f{b}_{ch}", name=f"f{b}_{ch}")
                        for ch in range(2)
                    ]
                    for b in range(B)
                ]
                NP = MT // 2
                for t in range(NP):
                    g = t // 2
                    ml = (t % 2) * 256
                    vt8 = vtp.tile([128, 2, B, C], f8, tag="vt8", name="vt8")
                    for b in range(B):
                        nc.sync.dma_start(
                            out=vt8[:, :, b, :],
                            in_=v_out[g, b, ml : ml + 256, :].rearrange(
                                "(two p) c -> p two c", p=128
                            ),
                        )
                    et = e2[t]
                    for par in range(2):
                        mt = 2 * t + par
                        r1h = r1a if mt < 16 else r1b
                        cb = (4 * mt) % 64
                        for b in range(B):
                            nc.vector.scalar_tensor_tensor(
                                out=et[:, par, b, :],
                                in0=rb_sb[mt],
                                scalar=r1h[:, cb + b : cb + b + 1],
                                in1=et[:, par, b, :],
                                op0=OP.add,
                                op1=OP.mult,
                            )
                    for b in range(B):
                        for ch in range(2):
                            nc.tensor.matmul(
                                fusion_ps[b][ch],
                                vt8[:, :, b, ch * 128 : (ch + 1) * 128],
                                et[:, :, b, :],
                                start=(t == 0),
                                stop=(t == NP - 1),
                                perf_mode=mybir.MatmulPerfMode.DoubleRow,
                            )

                # ---- stage raw fusion to fp8 conv layout ----------------
                for b in range(B):
                    for ch in range(2):
                        # scale by 1/4: TRN fp8e4 max-normal is 240 and
                        # |fusion| reaches ~275; the epilogue g absorbs the 4x
                        nc.scalar.activation(
                            out=ff8[b][:, ch, 1:9, 2:66],
                            in_=fusion_ps[b][ch].rearrange("p (r w) -> p r w", w=Wd),
                            func=AF.Copy,
                            scale=0.25,
                        )

        # =================================================================
        # phase E: ConvTranspose2d of the fusion branch (fp8 DoubleRow over
        # the two c-chunks), epilogue out = g_b * conv_f + staged conv_x
        # =================================================================
        with (
            tc.tile_pool(name="ostp", bufs=2) as ostp,
            tc.tile_pool(name="cps", bufs=1, space="PSUM") as cps,
        ):
            for py in range(2):
                ost = ostp.tile([128, B, 9, 2 * Wd], bf16, tag="ost", name="ost")
                for px in range(2):
                    pss = [
                        cps.tile([128, NOUT], f32, tag=f"cps{b}", name=f"cps{b}")
                        for b in range(B)
                    ]
                    taps = [
                        (ky, kx)
                        for ky in (py, py + 2)
                        for kx in (px, px + 2)
                    ]
                    for ti, (ky, kx) in enumerate(taps):
                        ro = (py + ky) // 2 - py
                        ww = (px + kx) // 2 - 1
                        for b in range(B):
                            fp = ff8[b]
                            nc.tensor.matmul(
                                pss[b][:, 0:512],
                                wco_pair(ky, kx),
                                fp[:, :, ro : ro + 8, 2 + ww : 66 + ww],
                                start=(ti == 0),
                                stop=(ti == len(taps) - 1),
                                perf_mode=mybir.MatmulPerfMode.DoubleRow,
                            )
                            nc.tensor.matmul(
                                pss[b][:, 512:NOUT],
                                wco_pair(ky, kx),
                                fp[:, :, ro + 8, 2 + ww : 66 + ww],
                                start=(ti == 0),
                                stop=(ti == len(taps) - 1),
                                perf_mode=mybir.MatmulPerfMode.DoubleRow,
                            )
                    for b in range(B):
                        ov = ost[:, b].rearrange("p j (w q) -> p j w q", q=2)[
                            :, :, :, px
                        ]
                        psv = pss[b].rearrange("p (j w) -> p j w", w=Wd)
                        nc.vector.scalar_tensor_tensor(
                            out=ov,
                            in0=psv,
                            scalar=g_bcast[:, b : b + 1],
                            in1=stg[:, py, px, b],
                            op0=OP.mult,
                            op1=OP.add,
                        )
                for b in range(B):
                    nc.sync.dma_start(
                        out=out_p[b].rearrange("c (j t) w -> c j t w", t=2)[
                            :, :, 1 - py, :
                        ],
                        in_=ost[:, b],
                    )

    nc.finalize()
    return nc


# ---------------------------------------------------------------------------
# host side
# ---------------------------------------------------------------------------
def _host_prep(x, wq, bq, wv, bv, w_adj1, b_adj1, w_adj2, b_adj2, gamma, w_co, b_co):
    import ml_dtypes

    bf16 = ml_dtypes.bfloat16
    x = np.asarray(x, np.float32).reshape(B, C, HW)
    xpad = np.zeros((B, C, HW + 4), np.float32)
    xpad[:, :, 2 : 2 + HW] = x

    wqT = np.ascontiguousarray(np.asarray(wq, np.float32).T)  # [C, C]
    wvT = np.ascontiguousarray(np.asarray(wv, np.float32).T)

    # grouped conv -> block-diagonal [3, 256, 32]
    w1 = np.zeros((3, C, CR), np.float32)
    wa1 = np.asarray(w_adj1, np.float32)  # [32, 8, 3]
    for g in range(CR):
        w1[:, g * 8 : (g + 1) * 8, g] = wa1[g].T  # [8,3] -> [3,8]

    # conv2 with output channels permuted to [query(32) | key(32)]
    wa2 = np.asarray(w_adj2, np.float32)  # [64, 32, 3]
    perm = np.concatenate([np.arange(0, 64, 2), np.arange(1, 64, 2)])
    w2 = np.ascontiguousarray(wa2[perm].transpose(2, 1, 0))  # [3, 32, 64]
    b2p = np.asarray(b_adj2, np.float32)[perm]

    # convT weights: flip, swap I/O -> [ky, kx, c_in, c_out] -> [32,128,128]
    wt = np.flip(np.asarray(w_co, np.float32), (2, 3)).transpose(1, 0, 2, 3)
    wco = np.ascontiguousarray(
        wt.transpose(2, 3, 1, 0).reshape(4, 4, 2, 128, 128).reshape(32, 128, 128)
    ).astype(bf16)

    # const pack (mask differs per core; rest shared)
    cbase = np.zeros((128, CPCOLS), np.float32)
    for k in range(2):
        cbase[:, OFF_WQ + k * 256 : OFF_WQ + (k + 1) * 256] = wqT[
            k * 128 : (k + 1) * 128, :
        ]
        cbase[:, OFF_WV + k * 256 : OFF_WV + (k + 1) * 256] = wvT[
            k * 128 : (k + 1) * 128, :
        ]
    for t in range(3):
        for k in range(2):
            o = OFF_W1 + (t * 2 + k) * CR
            cbase[:, o : o + CR] = w1[t, k * 128 : (k + 1) * 128, :]
        cbase[0:CR, OFF_W2 + t * 64 : OFF_W2 + (t + 1) * 64] = w2[t]
    cbase[:, OFF_BVB : OFF_BVB + C] = np.asarray(bv, np.float32)[None, :]

    # f32 pack: bq k0/k1, b1, b2(perm), bco, gamma
    fpack = np.zeros((128, 8), np.float32)
    bqf = np.asarray(bq, np.float32)
    fpack[:, 0] = bqf[0:128]
    fpack[:, 1] = bqf[128:256]
    fpack[0:CR, 2] = np.asarray(b_adj1, np.float32)
    fpack[0 : 2 * CR, 3] = b2p
    fpack[:, 4] = np.asarray(b_co, np.float32)
    fpack[0, 5] = np.asarray(gamma, np.float32).reshape(-1)[0]
    fpack[:, 6] = -2.0
    fpack = np.ascontiguousarray(fpack)

    in_maps = []
    for i in range(NCORES):
        n0 = i * NL
        xsl = xpad[:, :, n0 : n0 + XW]  # [B, C, XW]
        xpk = np.ascontiguousarray(
            xsl.reshape(B, 2, 128, XW).transpose(2, 0, 1, 3).astype(bf16)
        )
        j = np.arange(XW)
        valid = ((n0 - 2 + j) >= 0) & ((n0 - 2 + j) < HW)
        cpk = cbase.copy()
        cpk[:, OFF_MASK : OFF_MASK + XW] = valid.astype(np.float32)[None, :]
        in_maps.append(
            dict(
                cpack=np.ascontiguousarray(cpk.astype(bf16)),
                fpack=fpack,
                xpack=xpk,
                wco=wco,
            )
        )
    return in_maps


def _stitch(outs):
    full = np.zeros((B, C // 2, 2 * H, 2 * Wd), np.float32)
    for i in range(NCORES):
        y0 = 16 * i - 1
        lo = max(0, y0)
        hi = min(2 * H, y0 + OUTROWS)
        full[:, :, lo:hi, :] += np.asarray(
            outs[i][:, :, lo - y0 : hi - y0, :], np.float32
        )
    return full


def _get_nc():
    if "nc" not in _CACHE:
        _CACHE["nc"] = build_module()
    return _CACHE["nc"]


def run_spmd(in_maps, trace=False, **kw):
    from concourse.bass_utils import run_bass_kernel_spmd

    nc = _get_nc()
    return run_bass_kernel_spmd(
        nc, in_maps, core_ids=list(range(NCORES)), trace=trace, **kw
    )


def kernel(x, wq, bq, wv, bv, w_adj1, b_adj1, w_adj2, b_adj2, gamma, w_co, b_co):
    in_maps = _host_prep(
        x, wq, bq, wv, bv, w_adj1, b_adj1, w_adj2, b_adj2, gamma, w_co, b_co
    )
    res = run_spmd(in_maps)
    full = _stitch([r["out"] for r in res.results])
    # slab rows 0,1 carry no bias (the neighbor's rows complete them);
    # global row 0 has no neighbor, so add the bias here.
    full[:, :, 0, :] += np.asarray(b_co, np.float32)[None, :, None]
    return full.astype(np.float32)


# revision 34
# speedup vs baseline: 1.0052x; 1.0036x over previous
"""BidirectionalAttention Trainium2 Bass kernel — 8-core SPMD, v2.

Decomposition (same math as the verified baseline):
  q path : 1x1 conv (matmul) -> grouped conv1d k=3 -> conv1d k=3
  attn   : E = exp(q^T k); both softmaxes share one exp:
             attn_f + attn_b = E * (1/S0[n,m] + 1/S1[b,m])
             S0 = sum_b E  (batch softmax denom, axis=0)
             S1 = sum_n E  (row softmax denom, axis=1) -> two AllReduces
  fusion = value @ (attn_f+attn_b)^T scaled by gamma*mean(x_b), + x
  ConvTranspose2d(k=4,s=2,p=1) via the 4-subkernel parity decomposition,
  18-row output slabs with additive 2-row seams stitched on the host.

Performance structure (471us baseline -> ~352us):
  - K/Q/V and E in fp8e4 (TRN f8 max-normal is 240: exp carries a -2 bias
    so E' <= ~130; raw fusion is staged at 1/4 scale).  The attention
    branch output is scaled by gamma*mean(x) ~ 1e-4, so fp8 there moves
    the final output by ~1e-4 relative (verified by amplifying gamma
    3000x: fusion-dominant rel err ~5e-2, i.e. fp8-grade fidelity).
  - Warm-up 128-byte AllGather at t=0 absorbs the ~35us cross-core
    NEFF-start skew under phase A (the ncfw entry barrier otherwise
    stalls the first real collective).
  - One K AllGather (64KB) for all 4 batches; the 512KB V AllGather is
    pinned BEHIND it on the CC queue via tile_wait_until (phase C is
    gated on K; the scheduler otherwise reorders them).
  - ConvTranspose split by linearity: convT(g*s*fusion + x) =
    g*s*convT(fusion) + convT(x).  convT(x) runs in the collective
    dead-zone before phase C (PE stays warm) into an SBUF stage that
    also carries the bias; convT(fusion) uses fp8 DoubleRow over the
    two c-chunk k-tiles in phase E; epilogue = one scalar_tensor_tensor
    per (py,px,b): out = g_b * conv_f + stage.
  - Phase C: S1 partials via per-batch exp accum_out on Scalar for
    3-of-8 tiles, merged exp + DVE reduce otherwise (queue balance);
    S0 add-tree mostly on GpSimd; 1/S0 cached bf16 for phase D.
  - Phase D: A = (1/S0 + 1/S1) * E' in-place as one scalar_tensor_tensor
    per (m-pair, par, b); fusion matmuls are fp8 DoubleRow over m-tile
    pairs (rhs [128, 2, 512]), halving PE stream time.
  - Coalesced DMAs (one const pack, one x pack, per-b V stages, one wco
    load, per-b output DMAs); bf16 output, stitched on the host.
"""

import numpy as np

B = 4
C = 256
H = 64
Wd = 64
HW = H * Wd            # 4096
CR = 32                # C // 8
NCORES = 8
NL = HW // NCORES      # 512 owned attention rows (n) per core
HL = H // NCORES       # 8 owned image rows per core
MT = HW // 128         # 32 m-tiles of 128
XW = NL + 4            # x slab width (n halo +-2 for the two k=3 convs)
Q2W = NL + 2           # q2 width (halo +-1 for conv2)
ROWW = 68              # fusion_pad row width: [0,1]=zero, 2..65 data, [66,67]=zero
OUTROWS = 2 * HL + 2   # 18 output rows per core (2-row overlaps, host-stitched)

# const-pack column offsets (bf16 elements)
OFF_WQ = 0             # [2, 256]
OFF_WV = 512           # [2, 256]
OFF_W1 = 1024          # [3, 2, 32]
OFF_MASK = 1216        # [516]
OFF_BVB = 1732         # [256]
OFF_W2 = 1988          # rows 0:32, [3, 64]
CPCOLS = 2180

_CACHE = {}


# ---------------------------------------------------------------------------
# device module
# ---------------------------------------------------------------------------
def build_module():
    from contextlib import ExitStack

    import concourse.bass as bass
    import concourse.mybir as mybir
    from concourse import bacc
    from concourse.tile import TileContext

    f32 = mybir.dt.float32
    bf16 = mybir.dt.bfloat16
    f8 = mybir.dt.float8e4
    AF = mybir.ActivationFunctionType
    OP = mybir.AluOpType
    AX = mybir.AxisListType

    nc = bacc.Bacc(num_devices=NCORES)
    RG = [list(range(NCORES))]

    # ---- parameters (per-core) -------------------------------------------
    cpack_p = nc.declare_dram_parameter("cpack", [128, CPCOLS], bf16, isOutput=False)
    fpack_p = nc.declare_dram_parameter("fpack", [128, 8], f32, isOutput=False)
    xpack_p = nc.declare_dram_parameter("xpack", [128, B, 2, XW], bf16, isOutput=False)
    wco_p = nc.declare_dram_parameter("wco", [32, 128, 128], bf16, isOutput=False)
    out_p = nc.declare_dram_parameter(
        "out", [B, C // 2, OUTROWS, 2 * Wd], bf16, isOutput=True
    )

    with TileContext(nc) as tc, ExitStack() as ctx:
        # ---- long-lived pools -------------------------------------------
        const = ctx.enter_context(tc.tile_pool(name="const", bufs=1))
        xpool = ctx.enter_context(tc.tile_pool(name="xp", bufs=1))
        qkv = ctx.enter_context(tc.tile_pool(name="qkv", bufs=1))
        fpool = ctx.enter_context(tc.tile_pool(name="fp", bufs=1))
        dram = ctx.enter_context(tc.tile_pool(name="dram", bufs=1, space="DRAM"))

        # ---- DRAM bounce buffers ----------------------------------------
        k_in = dram.tile([B, CR, NL], f8, tag="k_in", name="k_in")
        k_out = dram.tile(
            [NCORES, B, CR, NL], f8, tag="k_out", name="k_out"
        )
        v_in = dram.tile([B, NL, C], f8, tag="v_in", name="v_in")
        v_out = dram.tile(
            [NCORES, B, NL, C], f8, tag="v_out", name="v_out"
        )
        ar1_in = dram.tile([128, 64], f32, tag="ar1_in", name="ar1_in")
        ar1_out = dram.tile(
            [128, 64], f32, tag="ar1_out", name="ar1_out"
        )
        ar2_in = dram.tile([128, 72], f32, tag="ar2_in", name="ar2_in")
        ar2_out = dram.tile(
            [128, 72], f32, tag="ar2_out", name="ar2_out"
        )
        g_dram = dram.tile([1, B], f32, tag="g_dram", name="g_dram")
        warm_in = dram.tile([1, 4], f32, tag="warm_in", name="warm_in")
        warm_out = dram.tile([NCORES, 4], f32, tag="warm_out", name="warm_out")

        # warm-up rendezvous: absorb the cross-core NEFF-start skew under
        # phase A instead of paying it at the first real collective
        with tc.high_priority():
            nc.gpsimd.collective_compute(
                "AllGather", OP.bypass, replica_groups=RG,
                ins=[warm_in[:, :]], outs=[warm_out[:, :]],
            )

        # ---- persistent SBUF state --------------------------------------
        fpk = const.tile([128, 8], f32, tag="fpk", name="fpk")
        nc.sync.dma_start(out=fpk, in_=fpack_p[:, :])
        xt = xpool.tile([128, B, 2, XW], bf16, tag="xt", name="xt")
        nc.sync.dma_start(out=xt, in_=xpack_p[:, :, :, :])

        s1p = qkv.tile([128, 136], f32, tag="s1p", name="s1p")
        Q_all = qkv.tile([128, NL], f8, tag="Q", name="Q")
        K_all = qkv.tile([128, HW], f8, tag="K", name="K")
        r1a = qkv.tile([128, 64], bf16, tag="r1a", name="r1a")  # 1/S1, mt<16
        r1b = qkv.tile([128, 64], bf16, tag="r1b", name="r1b")  # 1/S1, mt>=16
        g_bcast = qkv.tile([128, B], f32, tag="gbc", name="gbc")
        a1o = qkv.tile([128, 64], f32, tag="a1o", name="a1o")
        a2o = qkv.tile([128, 72], f32, tag="a2o", name="a2o")

        wt = const.tile([128, 32, 128], bf16, tag="wt", name="wt")
        nc.sync.dma_start(out=wt, in_=wco_p.rearrange("t p co -> p t co"))

        def wco_v(ky, kx, k):
            return wt[:, ky * 8 + kx * 2 + k, :]

        wt8 = const.tile([128, 32, 128], f8, tag="wt8", name="wt8")
        nc.scalar.copy(out=wt8, in_=wt)

        def wco_pair(ky, kx):
            return wt8[:, ky * 8 + kx * 2 : ky * 8 + kx * 2 + 2, :]

        # x in ConvT layout (halo rows/cols zero) and the staged convT(x)+bias
        fpx = [
            [
                fpool.tile([128, 10, ROWW], bf16, tag=f"fpx{b}_{ch}", name=f"fpx{b}_{ch}")
                for ch in range(2)
            ]
            for b in range(B)
        ]
        stg = fpool.tile([128, 2, 2, B, 9, Wd], bf16, tag="stg", name="stg")
        for b in range(B):
            for ch in range(2):
                nc.gpsimd.memset(fpx[b][ch], 0.0)
                nc.scalar.copy(
                    out=fpx[b][ch][:, 1:9, 2:66],
                    in_=xt[:, b, ch, 2 : 2 + NL].rearrange("p (r w) -> p r w", w=Wd),
                )

        def bq_v(k):
            return fpk[:, k : k + 1]

        b1_v = fpk[0:CR, 2:3]
        b2q_v = fpk[0:CR, 3:4]
        b2k_v = fpk[CR : 2 * CR, 3:4]
        bco_v = fpk[:, 4:5]
        gm_v = fpk[0:1, 5:6]
        nege2_v = fpk[:, 6:7]  # -2.0 exp bias (fp8 range)

        # =================================================================
        # phases A (q path) + B (value) under the scoped const pack
        # =================================================================
        with (
            tc.tile_pool(name="cpA", bufs=1) as cpA,
            tc.tile_pool(name="qtmp", bufs=2) as qtmp,
            tc.tile_pool(name="qps", bufs=2, space="PSUM") as qps,
            tc.tile_pool(name="q2ps", bufs=1, space="PSUM") as q2ps,
            tc.tile_pool(name="q3ps", bufs=1, space="PSUM") as q3ps,
            tc.tile_pool(name="vps", bufs=1, space="PSUM") as vps,
            tc.tile_pool(name="vst", bufs=2) as vst,
        ):
            cp = cpA.tile([128, CPCOLS], bf16, tag="cp", name="cp")
            nc.sync.dma_start(out=cp, in_=cpack_p[:, :])

            def wq_v(k):
                return cp[:, OFF_WQ + k * 256 : OFF_WQ + (k + 1) * 256]

            def wv_v(k):
                return cp[:, OFF_WV + k * 256 : OFF_WV + (k + 1) * 256]

            def w1_v(t, k):
                o = OFF_W1 + (t * 2 + k) * CR
                return cp[:, o : o + CR]

            def w2_v(t):
                o = OFF_W2 + t * 64
                return cp[0:CR, o : o + 64]

            mask_v = cp[:, OFF_MASK : OFF_MASK + XW]
            bvb_v = cp[:, OFF_BVB : OFF_BVB + C]

            # x partial sums (for gamma*mean(x)) at s1p cols 128 + b*2 + k
            for b in range(B):
                for k in range(2):
                    cc = 128 + b * 2 + k
                    nc.vector.tensor_reduce(
                        out=s1p[:, cc : cc + 1],
                        in_=xt[:, b, k, 2 : 2 + NL],
                        axis=AX.X,
                        op=OP.add,
                    )

            # ---- phase A: q path per batch ------------------------------
            for b in range(B):
                q1_sb = []
                for mtile in range(2):
                    ps = qps.tile([128, XW], f32, tag="q1ps", name="q1ps")
                    for k in range(2):
                        for lo, hi in ((0, 512), (512, XW)):
                            nc.tensor.matmul(
                                ps[:, lo:hi],
                                wq_v(k)[:, mtile * 128 : (mtile + 1) * 128],
                                xt[:, b, k, lo:hi],
                                start=(k == 0),
                                stop=(k == 1),
                            )
                    q1 = qtmp.tile([128, XW], bf16, tag=f"q1_{mtile}", name=f"q1_{mtile}")
                    nc.scalar.activation(
                        out=q1, in_=ps, func=AF.Identity, bias=bq_v(mtile)
                    )
                    nc.vector.tensor_mul(q1, q1, mask_v)
                    q1_sb.append(q1)

                ps2 = q2ps.tile([CR, Q2W], f32, tag="q2ps", name="q2ps")
                for t in range(3):
                    for k in range(2):
                        st = t == 0 and k == 0
                        sp = t == 2 and k == 1
                        for lo, hi in ((0, 512), (512, Q2W)):
                            nc.tensor.matmul(
                                ps2[:, lo:hi],
                                w1_v(t, k),
                                q1_sb[k][:, lo + t : hi + t],
                                start=st,
                                stop=sp,
                            )
                q2 = qtmp.tile([CR, Q2W], bf16, tag="q2", name="q2")
                nc.scalar.activation(out=q2, in_=ps2, func=AF.Identity, bias=b1_v)
                nc.vector.tensor_mul(q2, q2, mask_v[:CR, 1 : 1 + Q2W])

                ps3 = q3ps.tile([2 * CR, NL], f32, tag="q3ps", name="q3ps")
                for t in range(3):
                    nc.tensor.matmul(
                        ps3,
                        w2_v(t),
                        q2[:, t : t + NL],
                        start=(t == 0),
                        stop=(t == 2),
                    )
                q3 = qtmp.tile([2 * CR, NL], f8, tag="q3", name="q3")
                nc.scalar.activation(
                    out=q3, in_=ps3, func=AF.Identity, bias=fpk[0 : 2 * CR, 3:4]
                )
                nc.sync.dma_start(
                    out=Q_all[CR * b : CR * (b + 1), :], in_=q3[0:CR, :]
                )
                nc.sync.dma_start(out=k_in[b], in_=q3[CR : 2 * CR, :])

            # single K AllGather for all 4 batches; high priority so the
            # scheduler keeps it AHEAD of the (bigger) V AllGather on the CC
            # queue -- phase C is gated on K
            with tc.high_priority(offset=1000):
                nc.gpsimd.collective_compute(
                    "AllGather",
                    OP.bypass,
                    replica_groups=RG,
                    ins=[k_in[:, :, :]],
                    outs=[k_out[:, :, :, :]],
                )

            # ---- phase B: value^T shards, fp8 ---------------------------
            for b in range(B):
                vstage = vst.tile([128, 4, C], f8, tag="vstage", name="vstage")
                for ms in range(4):
                    psv = vps.tile([128, C], f32, tag="vpsm", name="vpsm")
                    for k in range(2):
                        nc.tensor.matmul(
                            psv,
                            xt[:, b, k, 2 + ms * 128 : 2 + (ms + 1) * 128],
                            wv_v(k),
                            start=(k == 0),
                            stop=(k == 1),
                        )
                    nc.vector.tensor_add(vstage[:, ms, :], psv, bvb_v)
                nc.sync.dma_start(
                    out=v_in[b].rearrange("(ms p) c -> p ms c", p=128), in_=vstage
                )

            # assemble K_all from the gathered shards (per-b: the SBUF dst
            # must keep a single partition dim)
            for b in range(B):
                nc.sync.dma_start(
                    out=K_all[CR * b : CR * (b + 1), :].rearrange(
                        "c (g m) -> c g m", g=NCORES
                    ),
                    in_=k_out[:, b].rearrange("g c m -> c g m"),
                )

        # force the V AllGather BEHIND the K AllGather on the CC queue
        # (phase C is gated on K; the scheduler otherwise reorders them)
        with tc.tile_wait_until(0.06):
            nc.gpsimd.collective_compute(
                "AllGather",
                OP.bypass,
                replica_groups=RG,
                ins=[v_in[:, :, :]],
                outs=[v_out[:, :, :, :]],
            )

        # =================================================================
        # conv-x: ConvTranspose of the residual x, staged to SBUF (+bias).
        # Runs in the collective dead-zone; keeps the PE warm before C.
        # =================================================================
        NOUT = 9 * Wd  # 576 spatial outputs per (b, py, px)
        with tc.tile_pool(name="cvx", bufs=1, space="PSUM") as cvx:
            for py in range(2):
                for px in range(2):
                    psx = [
                        cvx.tile([128, NOUT], f32, tag=f"cvx{b}", name=f"cvx{b}")
                        for b in range(B)
                    ]
                    taps = [
                        (ky, kx, k)
                        for ky in (py, py + 2)
                        for kx in (px, px + 2)
                        for k in range(2)
                    ]
                    for ti, (ky, kx, k) in enumerate(taps):
                        ro = (py + ky) // 2 - py
                        ww = (px + kx) // 2 - 1
                        for b in range(B):
                            fp = fpx[b][k]
                            nc.tensor.matmul(
                                psx[b][:, 0:512],
                                wco_v(ky, kx, k),
                                fp[:, ro : ro + 8, 2 + ww : 66 + ww],
                                start=(ti == 0),
                                stop=(ti == len(taps) - 1),
                            )
                            nc.tensor.matmul(
                                psx[b][:, 512:NOUT],
                                wco_v(ky, kx, k),
                                fp[:, ro + 8, 2 + ww : 66 + ww],
                                start=(ti == 0),
                                stop=(ti == len(taps) - 1),
                            )
                    for b in range(B):
                        sv = stg[:, py, px, b]
                        pv = psx[b].rearrange("p (j w) -> p j w", w=Wd)
                        nc.scalar.activation(
                            out=sv[:, 1:9, :], in_=pv[:, 1:9, :],
                            func=AF.Identity, bias=bco_v,
                        )
                        nc.scalar.activation(
                            out=sv[:, 0:1, :], in_=pv[:, 0:1, :], func=AF.Copy,
                        )

        # =================================================================
        # phases C (QK + exp + denominators) and D (scale + fusion matmul)
        # =================================================================
        with tc.tile_pool(name="work", bufs=1) as work:
            # E in fp8e4 (exp bias -2 keeps E' <= ~130 < 448), stored as
            # m-tile PAIRS [128, 2, B, NL] for DoubleRow fusion matmuls
            e2 = [
                work.tile([128, 2, B, NL], f8, tag=f"e{t}", name=f"e{t}")
                for t in range(MT // 2)
            ]
            rb_sb = [
                work.tile([128, NL], bf16, tag=f"rb{mt}", name=f"rb{mt}")
                for mt in range(MT)
            ]

            with (
                tc.tile_pool(name="qk", bufs=2, space="PSUM") as qk,
                tc.tile_pool(name="sc", bufs=2) as sc,
            ):
                for mt in range(MT):
                    ps4 = qk.tile([128, B, NL], f32, tag="e4ps", name="e4ps")
                    for b in range(B):
                        nc.tensor.matmul(
                            ps4[:, b, :],
                            K_all[CR * b : CR * (b + 1), mt * 128 : (mt + 1) * 128],
                            Q_all[CR * b : CR * (b + 1), :],
                            start=True,
                            stop=True,
                            tile_position=(CR * b, 0),
                        )
                    ev = e2[mt // 2][:, mt % 2]  # [128, B, NL] view
                    # S1 partials: Scalar (per-b exp accum_out) for the
                    # tiles feeding the ARs (so AR inputs land with the exp,
                    # not at the DVE queue tail); DVE reduce for the middle
                    if mt % 8 < 3:
                        for b in range(B):
                            col = 4 * mt + b
                            nc.scalar.activation(
                                out=ev[:, b, :],
                                in_=ps4[:, b, :],
                                func=AF.Exp,
                                bias=nege2_v,
                                accum_out=s1p[:, col : col + 1],
                            )
                    else:
                        nc.scalar.activation(out=ev, in_=ps4, func=AF.Exp, bias=nege2_v)
                        nc.vector.tensor_reduce(
                            out=s1p[:, 4 * mt : 4 * mt + 4],
                            in_=ev,
                            axis=AX.X,
                            op=OP.add,
                        )
                    # S0 = sum_b E: add tree, mostly on GpSimd
                    s0f = sc.tile([128, NL], f32, tag="s0f", name="s0f")
                    if mt % 4 == 0:
                        t2 = sc.tile([128, 2, NL], bf16, tag="t2", name="t2")
                        nc.vector.tensor_add(t2, ev[:, 0:2, :], ev[:, 2:4, :])
                        nc.vector.tensor_add(s0f, t2[:, 0, :], t2[:, 1, :])
                    else:
                        s01 = sc.tile([128, NL], bf16, tag="s01", name="s01")
                        s23 = sc.tile([128, NL], bf16, tag="s23", name="s23")
                        nc.gpsimd.tensor_add(s01, ev[:, 0, :], ev[:, 1, :])
                        nc.gpsimd.tensor_add(s23, ev[:, 2, :], ev[:, 3, :])
                        nc.gpsimd.tensor_add(s0f, s01, s23)
                    rf = sc.tile([128, NL], f32, tag="rf", name="rf")
                    nc.vector.reciprocal_approx_fast(out=rf, in_=s0f)
                    if mt % 2 == 0:
                        nc.scalar.copy(out=rb_sb[mt], in_=rf)
                    else:
                        nc.vector.tensor_copy(rb_sb[mt], rf)

                    if mt == MT // 2 - 1:
                        nc.sync.dma_start(out=ar1_in[:, :], in_=s1p[:, 0:64])
                        nc.gpsimd.collective_compute(
                            "AllReduce", OP.add, replica_groups=RG,
                            ins=[ar1_in[:, :]], outs=[ar1_out[:, :]],
                        )
                        nc.sync.dma_start(out=a1o, in_=ar1_out[:, :])
                        r1f = sc.tile([128, 64], f32, tag="r1f", name="r1f")
                        nc.vector.reciprocal_approx_fast(out=r1f, in_=a1o)
                        nc.vector.tensor_copy(r1a, r1f)

                # second AR half: S1 cols 64..128 plus the x sums
                nc.sync.dma_start(out=ar2_in[:, 0:64], in_=s1p[:, 64:128])
                nc.sync.dma_start(out=ar2_in[:, 64:72], in_=s1p[:, 128:136])
                nc.gpsimd.collective_compute(
                    "AllReduce", OP.add, replica_groups=RG,
                    ins=[ar2_in[:, :]], outs=[ar2_out[:, :]],
                )
                nc.sync.dma_start(out=a2o, in_=ar2_out[:, :])
                r2f = sc.tile([128, 64], f32, tag="r2f", name="r2f")
                nc.vector.reciprocal_approx_fast(out=r2f, in_=a2o[:, 0:64])
                nc.vector.tensor_copy(r1b, r2f)

                # g_bcast[p, b] = gamma * mean(x[b])
                xps = sc.tile([1, 8], f32, tag="xps", name="xps")
                nc.gpsimd.tensor_reduce(
                    out=xps, in_=a2o[:, 64:72], axis=AX.C, op=OP.add
                )
                xv = xps.rearrange("p (b k) -> p b k", b=B)
                g0 = sc.tile([1, B], f32, tag="g0", name="g0")
                nc.vector.tensor_add(g0, xv[:, :, 0], xv[:, :, 1])
                nc.vector.tensor_scalar(
                    out=g0,
                    in0=g0,
                    scalar1=gm_v,
                    scalar2=float(4.0 / (C * HW)),
                    op0=OP.mult,
                    op1=OP.mult,
                )
                nc.sync.dma_start(out=g_dram[:, :], in_=g0)
                nc.sync.dma_start(
                    out=g_bcast,
                    in_=bass.AP(
                        tensor=g_dram.tensor,
                        offset=g_dram.offset,
                        ap=[[0, 128], [1, B]],
                    ),
                )

            # raw fusion in fp8 (|fusion| ~ 13 << 448); gamma*mean(x) is
            # applied in the phase-E epilogue (convT is linear, g is a
            # per-batch scalar)
            ff8 = [
                work.tile([128, 2, 10, ROWW], f8, tag=f"ff8{b}", name=f"ff8{b}")
                for b in range(B)
            ]
            for b in range(B):
                nc.gpsimd.memset(ff8[b], 0.0)

            # ---- phase D: A = E*(1/S0 + 1/S1) in place; fusion matmuls --
            with (
                tc.tile_pool(name="fus", bufs=1, space="PSUM") as fus,
                tc.tile_pool(name="vtp", bufs=4) as vtp,
            ):
                fusion_ps = [
                    [
                        fus.tile([128, NL], f32, tag=f"f{b}_{ch}", name=f"f{b}_{ch}")
                        for ch in range(2)
                    ]
                    for b in range(B)
                ]
                NP = MT // 2
                for t in range(NP):
                    g = t // 2
                    ml = (t % 2) * 256
                    vt8 = vtp.tile([128, 2, B, C], f8, tag="vt8", name="vt8")
                    for b in range(B):
                        nc.sync.dma_start(
                            out=vt8[:, :, b, :],
                            in_=v_out[g, b, ml : ml + 256, :].rearrange(
                                "(two p) c -> p two c", p=128
                            ),
                        )
                    et = e2[t]
                    for par in range(2):
                        mt = 2 * t + par
                        r1h = r1a if mt < 16 else r1b
                        cb = (4 * mt) % 64
                        for b in range(B):
                            nc.vector.scalar_tensor_tensor(
                                out=et[:, par, b, :],
                                in0=rb_sb[mt],
                                scalar=r1h[:, cb + b : cb + b + 1],
                                in1=et[:, par, b, :],
                                op0=OP.add,
                                op1=OP.mult,
                            )
                    for b in range(B):
                        for ch in range(2):
                            nc.tensor.matmul(
                                fusion_ps[b][ch],
                                vt8[:, :, b, ch * 128 : (ch + 1) * 128],
                                et[:, :, b, :],
                                start=(t == 0),
                                stop=(t == NP - 1),
                                perf_mode=mybir.MatmulPerfMode.DoubleRow,
                            )

                # ---- stage raw fusion to fp8 conv layout ----------------
                for b in range(B):
                    for ch in range(2):
                        # scale by 1/4: TRN fp8e4 max-normal is 240 and
                        # |fusion| reaches ~275; the epilogue g absorbs the 4x
                        nc.scalar.activation(
                            out=ff8[b][:, ch, 1:9, 2:66],
                            in_=fusion_ps[b][ch].rearrange("p (r w) -> p r w", w=Wd),
                            func=AF.Copy,
                            scale=0.25,
                        )

        # =================================================================
        # phase E: ConvTranspose2d of the fusion branch (fp8 DoubleRow over
        # the two c-chunks), epilogue out = g_b * conv_f + staged conv_x
        # =================================================================
        with (
            tc.tile_pool(name="ostp", bufs=2) as ostp,
            tc.tile_pool(name="cps", bufs=1, space="PSUM") as cps,
        ):
            for py in range(2):
                ost = ostp.tile([128, B, 9, 2 * Wd], bf16, tag="ost", name="ost")
                for px in range(2):
                    pss = [
                        cps.tile([128, NOUT], f32, tag=f"cps{b}", name=f"cps{b}")
                        for b in range(B)
                    ]
                    taps = [
                        (ky, kx)
                        for ky in (py, py + 2)
                        for kx in (px, px + 2)
                    ]
                    for ti, (ky, kx) in enumerate(taps):
                        ro = (py + ky) // 2 - py
                        ww = (px + kx) // 2 - 1
                        for b in range(B):
                            fp = ff8[b]
                            nc.tensor.matmul(
                                pss[b][:, 0:512],
                                wco_pair(ky, kx),
                                fp[:, :, ro : ro + 8, 2 + ww : 66 + ww],
                                start=(ti == 0),
                                stop=(ti == len(taps) - 1),
                                perf_mode=mybir.MatmulPerfMode.DoubleRow,
                            )
                            nc.tensor.matmul(
                                pss[b][:, 512:NOUT],
                                wco_pair(ky, kx),
                                fp[:, :, ro + 8, 2 + ww : 66 + ww],
                                start=(ti == 0),
                                stop=(ti == len(taps) - 1),
                                perf_mode=mybir.MatmulPerfMode.DoubleRow,
                            )
                    for b in range(B):
                        ov = ost[:, b].rearrange("p j (w q) -> p j w q", q=2)[
                            :, :, :, px
                        ]
                        psv = pss[b].rearrange("p (j w) -> p j w", w=Wd)
                        nc.vector.scalar_tensor_tensor(
                            out=ov,
                            in0=psv,
                            scalar=g_bcast[:, b : b + 1],
                            in1=stg[:, py, px, b],
                            op0=OP.mult,
                            op1=OP.add,
                        )
                for b in range(B):
                    nc.sync.dma_start(
                        out=out_p[b].rearrange("c (j t) w -> c j t w", t=2)[
                            :, :, 1 - py, :
                        ],
                        in_=ost[:, b],
                    )

    nc.finalize()
    return nc


# ---------------------------------------------------------------------------
# host side
# ---------------------------------------------------------------------------
def _host_prep(x, wq, bq, wv, bv, w_adj1, b_adj1, w_adj2, b_adj2, gamma, w_co, b_co):
    import ml_dtypes

    bf16 = ml_dtypes.bfloat16
    x = np.asarray(x, np.float32).reshape(B, C, HW)
    xpad = np.zeros((B, C, HW + 4), np.float32)
    xpad[:, :, 2 : 2 + HW] = x

    wqT = np.ascontiguousarray(np.asarray(wq, np.float32).T)  # [C, C]
    wvT = np.ascontiguousarray(np.asarray(wv, np.float32).T)

    # grouped conv -> block-diagonal [3, 256, 32]
    w1 = np.zeros((3, C, CR), np.float32)
    wa1 = np.asarray(w_adj1, np.float32)  # [32, 8, 3]
    for g in range(CR):
        w1[:, g * 8 : (g + 1) * 8, g] = wa1[g].T  # [8,3] -> [3,8]

    # conv2 with output channels permuted to [query(32) | key(32)]
    wa2 = np.asarray(w_adj2, np.float32)  # [64, 32, 3]
    perm = np.concatenate([np.arange(0, 64, 2), np.arange(1, 64, 2)])
    w2 = np.ascontiguousarray(wa2[perm].transpose(2, 1, 0))  # [3, 32, 64]
    b2p = np.asarray(b_adj2, np.float32)[perm]

    # convT weights: flip, swap I/O -> [ky, kx, c_in, c_out] -> [32,128,128]
    wt = np.flip(np.asarray(w_co, np.float32), (2, 3)).transpose(1, 0, 2, 3)
    wco = np.ascontiguousarray(
        wt.transpose(2, 3, 1, 0).reshape(4, 4, 2, 128, 128).reshape(32, 128, 128)
    ).astype(bf16)

    # const pack (mask differs per core; rest shared)
    cbase = np.zeros((128, CPCOLS), np.float32)
    for k in range(2):
        cbase[:, OFF_WQ + k * 256 : OFF_WQ + (k + 1) * 256] = wqT[
            k * 128 : (k + 1) * 128, :
        ]
        cbase[:, OFF_WV + k * 256 : OFF_WV + (k + 1) * 256] = wvT[
            k * 128 : (k + 1) * 128, :
        ]
    for t in range(3):
        for k in range(2):
            o = OFF_W1 + (t * 2 + k) * CR
            cbase[:, o : o + CR] = w1[t, k * 128 : (k + 1) * 128, :]
        cbase[0:CR, OFF_W2 + t * 64 : OFF_W2 + (t + 1) * 64] = w2[t]
    cbase[:, OFF_BVB : OFF_BVB + C] = np.asarray(bv, np.float32)[None, :]

    # f32 pack: bq k0/k1, b1, b2(perm), bco, gamma
    fpack = np.zeros((128, 8), np.float32)
    bqf = np.asarray(bq, np.float32)
    fpack[:, 0] = bqf[0:128]
    fpack[:, 1] = bqf[128:256]
    fpack[0:CR, 2] = np.asarray(b_adj1, np.float32)
    fpack[0 : 2 * CR, 3] = b2p
    fpack[:, 4] = np.asarray(b_co, np.float32)
    fpack[0, 5] = np.asarray(gamma, np.float32).reshape(-1)[0]
    fpack[:, 6] = -2.0
    fpack = np.ascontiguousarray(fpack)

    in_maps = []
    for i in range(NCORES):
        n0 = i * NL
        xsl = xpad[:, :, n0 : n0 + XW]  # [B, C, XW]
        xpk = np.ascontiguousarray(
            xsl.reshape(B, 2, 128, XW).transpose(2, 0, 1, 3).astype(bf16)
        )
        j = np.arange(XW)
        valid = ((n0 - 2 + j) >= 0) & ((n0 - 2 + j) < HW)
        cpk = cbase.copy()
        cpk[:, OFF_MASK : OFF_MASK + XW] = valid.astype(np.float32)[None, :]
        in_maps.append(
            dict(
                cpack=np.ascontiguousarray(cpk.astype(bf16)),
                fpack=fpack,
                xpack=xpk,
                wco=wco,
            )
        )
    return in_maps


def _stitch(outs):
    full = np.zeros((B, C // 2, 2 * H, 2 * Wd), np.float32)
    for i in range(NCORES):
        y0 = 16 * i - 1
        lo = max(0, y0)
        hi = min(2 * H, y0 + OUTROWS)
        full[:, :, lo:hi, :] += np.asarray(
            outs[i][:, :, lo - y0 : hi - y0, :], np.float32
        )
    return full


def _get_nc():
    if "nc" not in _CACHE:
        _CACHE["nc"] = build_module()
    return _CACHE["nc"]


def run_spmd(in_maps, trace=False, **kw):
    from concourse.bass_utils import run_bass_kernel_spmd

    nc = _get_nc()
    return run_bass_kernel_spmd(
        nc, in_maps, core_ids=list(range(NCORES)), trace=trace, **kw
    )


def kernel(x, wq, bq, wv, bv, w_adj1, b_adj1, w_adj2, b_adj2, gamma, w_co, b_co):
    in_maps = _host_prep(
        x, wq, bq, wv, bv, w_adj1, b_adj1, w_adj2, b_adj2, gamma, w_co, b_co
    )
    res = run_spmd(in_maps)
    full = _stitch([r["out"] for r in res.results])
    # slab rows 0,1 carry no bias (the neighbor's rows complete them);
    # global row 0 has no neighbor, so add the bias here.
    full[:, :, 0, :] += np.asarray(b_co, np.float32)[None, :, None]
    return full.astype(np.float32)


# revision 35
# speedup vs baseline: 1.0095x; 1.0042x over previous
"""BidirectionalAttention Trainium2 Bass kernel — 8-core SPMD, v2.

Decomposition (same math as the verified baseline):
  q path : 1x1 conv (matmul) -> grouped conv1d k=3 -> conv1d k=3
  attn   : E = exp(q^T k); both softmaxes share one exp:
             attn_f + attn_b = E * (1/S0[n,m] + 1/S1[b,m])
             S0 = sum_b E  (batch softmax denom, axis=0)
             S1 = sum_n E  (row softmax denom, axis=1) -> two AllReduces
  fusion = value @ (attn_f+attn_b)^T scaled by gamma*mean(x_b), + x
  ConvTranspose2d(k=4,s=2,p=1) via the 4-subkernel parity decomposition,
  18-row output slabs with additive 2-row seams stitched on the host.

Performance structure (471us baseline -> ~352us):
  - K/Q/V and E in fp8e4 (TRN f8 max-normal is 240: exp carries a -2 bias
    so E' <= ~130; raw fusion is staged at 1/4 scale).  The attention
    branch output is scaled by gamma*mean(x) ~ 1e-4, so fp8 there moves
    the final output by ~1e-4 relative (verified by amplifying gamma
    3000x: fusion-dominant rel err ~5e-2, i.e. fp8-grade fidelity).
  - Warm-up 128-byte AllGather at t=0 absorbs the ~35us cross-core
    NEFF-start skew under phase A (the ncfw entry barrier otherwise
    stalls the first real collective).
  - One K AllGather (64KB) for all 4 batches; the 512KB V AllGather is
    pinned BEHIND it on the CC queue via tile_wait_until (phase C is
    gated on K; the scheduler otherwise reorders them).
  - ConvTranspose split by linearity: convT(g*s*fusion + x) =
    g*s*convT(fusion) + convT(x).  convT(x) runs in the collective
    dead-zone before phase C (PE stays warm) into an SBUF stage that
    also carries the bias; convT(fusion) uses fp8 DoubleRow over the
    two c-chunk k-tiles in phase E; epilogue = one scalar_tensor_tensor
    per (py,px,b): out = g_b * conv_f + stage.
  - Phase C: S1 partials via per-batch exp accum_out on Scalar for
    3-of-8 tiles, merged exp + DVE reduce otherwise (queue balance);
    S0 add-tree mostly on GpSimd; 1/S0 cached bf16 for phase D.
  - Phase D: A = (1/S0 + 1/S1) * E' in-place as one scalar_tensor_tensor
    per (m-pair, par, b); fusion matmuls are fp8 DoubleRow over m-tile
    pairs (rhs [128, 2, 512]), halving PE stream time.
  - Coalesced DMAs (one const pack, one x pack, per-b V stages, one wco
    load, per-b output DMAs); bf16 output, stitched on the host.
"""

import numpy as np

B = 4
C = 256
H = 64
Wd = 64
HW = H * Wd            # 4096
CR = 32                # C // 8
NCORES = 8
NL = HW // NCORES      # 512 owned attention rows (n) per core
HL = H // NCORES       # 8 owned image rows per core
MT = HW // 128         # 32 m-tiles of 128
XW = NL + 4            # x slab width (n halo +-2 for the two k=3 convs)
Q2W = NL + 2           # q2 width (halo +-1 for conv2)
ROWW = 68              # fusion_pad row width: [0,1]=zero, 2..65 data, [66,67]=zero
OUTROWS = 2 * HL + 2   # 18 output rows per core (2-row overlaps, host-stitched)

# const-pack column offsets (bf16 elements)
OFF_WQ = 0             # [2, 256]
OFF_WV = 512           # [2, 256]
OFF_W1 = 1024          # [3, 2, 32]
OFF_MASK = 1216        # [516]
OFF_BVB = 1732         # [256]
OFF_W2 = 1988          # rows 0:32, [3, 64]
CPCOLS = 2180

_CACHE = {}


# ---------------------------------------------------------------------------
# device module
# ---------------------------------------------------------------------------
def build_module():
    from contextlib import ExitStack

    import concourse.bass as bass
    import concourse.mybir as mybir
    from concourse import bacc
    from concourse.tile import TileContext

    f32 = mybir.dt.float32
    bf16 = mybir.dt.bfloat16
    f8 = mybir.dt.float8e4
    AF = mybir.ActivationFunctionType
    OP = mybir.AluOpType
    AX = mybir.AxisListType

    nc = bacc.Bacc(num_devices=NCORES)
    RG = [list(range(NCORES))]

    # ---- parameters (per-core) -------------------------------------------
    cpack_p = nc.declare_dram_parameter("cpack", [128, CPCOLS], bf16, isOutput=False)
    fpack_p = nc.declare_dram_parameter("fpack", [128, 8], f32, isOutput=False)
    xpack_p = nc.declare_dram_parameter("xpack", [128, B, 2, XW], bf16, isOutput=False)
    wco_p = nc.declare_dram_parameter("wco", [32, 128, 128], bf16, isOutput=False)
    out_p = nc.declare_dram_parameter(
        "out", [B, C // 2, OUTROWS, 2 * Wd], bf16, isOutput=True
    )

    with TileContext(nc) as tc, ExitStack() as ctx:
        # ---- long-lived pools -------------------------------------------
        const = ctx.enter_context(tc.tile_pool(name="const", bufs=1))
        xpool = ctx.enter_context(tc.tile_pool(name="xp", bufs=1))
        qkv = ctx.enter_context(tc.tile_pool(name="qkv", bufs=1))
        fpool = ctx.enter_context(tc.tile_pool(name="fp", bufs=1))
        dram = ctx.enter_context(tc.tile_pool(name="dram", bufs=1, space="DRAM"))

        # ---- DRAM bounce buffers ----------------------------------------
        k_in = dram.tile([B, CR, NL], f8, tag="k_in", name="k_in")
        k_out = dram.tile(
            [NCORES, B, CR, NL], f8, tag="k_out", name="k_out"
        )
        v_in = dram.tile([B, NL, C], f8, tag="v_in", name="v_in")
        v_out = dram.tile(
            [NCORES, B, NL, C], f8, tag="v_out", name="v_out"
        )
        ar1_in = dram.tile([128, 64], f32, tag="ar1_in", name="ar1_in")
        ar1_out = dram.tile(
            [128, 64], f32, tag="ar1_out", name="ar1_out"
        )
        ar2_in = dram.tile([128, 72], f32, tag="ar2_in", name="ar2_in")
        ar2_out = dram.tile(
            [128, 72], f32, tag="ar2_out", name="ar2_out"
        )
        g_dram = dram.tile([1, B], f32, tag="g_dram", name="g_dram")
        warm_in = dram.tile([1, 4], f32, tag="warm_in", name="warm_in")
        warm_out = dram.tile([NCORES, 4], f32, tag="warm_out", name="warm_out")

        # warm-up rendezvous: absorb the cross-core NEFF-start skew under
        # phase A instead of paying it at the first real collective
        with tc.high_priority():
            nc.gpsimd.collective_compute(
                "AllGather", OP.bypass, replica_groups=RG,
                ins=[warm_in[:, :]], outs=[warm_out[:, :]],
            )

        # ---- persistent SBUF state --------------------------------------
        fpk = const.tile([128, 8], f32, tag="fpk", name="fpk")
        nc.sync.dma_start(out=fpk, in_=fpack_p[:, :])
        xt = xpool.tile([128, B, 2, XW], bf16, tag="xt", name="xt")
        nc.sync.dma_start(out=xt, in_=xpack_p[:, :, :, :])

        s1p = qkv.tile([128, 136], f32, tag="s1p", name="s1p")
        Q_all = qkv.tile([128, NL], f8, tag="Q", name="Q")
        K_all = qkv.tile([128, HW], f8, tag="K", name="K")
        r1a = qkv.tile([128, 64], bf16, tag="r1a", name="r1a")  # 1/S1, mt<16
        r1b = qkv.tile([128, 64], bf16, tag="r1b", name="r1b")  # 1/S1, mt>=16
        g_bcast = qkv.tile([128, B], f32, tag="gbc", name="gbc")
        a1o = qkv.tile([128, 64], f32, tag="a1o", name="a1o")
        a2o = qkv.tile([128, 72], f32, tag="a2o", name="a2o")

        wt = const.tile([128, 32, 128], bf16, tag="wt", name="wt")
        nc.sync.dma_start(out=wt, in_=wco_p.rearrange("t p co -> p t co"))

        def wco_v(ky, kx, k):
            return wt[:, ky * 8 + kx * 2 + k, :]

        wt8 = const.tile([128, 32, 128], f8, tag="wt8", name="wt8")
        nc.scalar.copy(out=wt8, in_=wt)

        def wco_pair(ky, kx):
            return wt8[:, ky * 8 + kx * 2 : ky * 8 + kx * 2 + 2, :]

        # x in ConvT layout (halo rows/cols zero) and the staged convT(x)+bias
        fpx = [
            [
                fpool.tile([128, 10, ROWW], bf16, tag=f"fpx{b}_{ch}", name=f"fpx{b}_{ch}")
                for ch in range(2)
            ]
            for b in range(B)
        ]
        stg = fpool.tile([128, 2, 2, B, 9, Wd], bf16, tag="stg", name="stg")
        for b in range(B):
            for ch in range(2):
                nc.gpsimd.memset(fpx[b][ch], 0.0)
                nc.scalar.copy(
                    out=fpx[b][ch][:, 1:9, 2:66],
                    in_=xt[:, b, ch, 2 : 2 + NL].rearrange("p (r w) -> p r w", w=Wd),
                )

        def bq_v(k):
            return fpk[:, k : k + 1]

        b1_v = fpk[0:CR, 2:3]
        b2q_v = fpk[0:CR, 3:4]
        b2k_v = fpk[CR : 2 * CR, 3:4]
        bco_v = fpk[:, 4:5]
        gm_v = fpk[0:1, 5:6]
        nege2_v = fpk[:, 6:7]  # -2.0 exp bias (fp8 range)

        # =================================================================
        # phases A (q path) + B (value) under the scoped const pack
        # =================================================================
        with (
            tc.tile_pool(name="cpA", bufs=1) as cpA,
            tc.tile_pool(name="qtmp", bufs=2) as qtmp,
            tc.tile_pool(name="qps", bufs=2, space="PSUM") as qps,
            tc.tile_pool(name="q2ps", bufs=1, space="PSUM") as q2ps,
            tc.tile_pool(name="q3ps", bufs=1, space="PSUM") as q3ps,
            tc.tile_pool(name="vps", bufs=1, space="PSUM") as vps,
            tc.tile_pool(name="vst", bufs=2) as vst,
        ):
            cp = cpA.tile([128, CPCOLS], bf16, tag="cp", name="cp")
            nc.sync.dma_start(out=cp, in_=cpack_p[:, :])

            def wq_v(k):
                return cp[:, OFF_WQ + k * 256 : OFF_WQ + (k + 1) * 256]

            def wv_v(k):
                return cp[:, OFF_WV + k * 256 : OFF_WV + (k + 1) * 256]

            def w1_v(t, k):
                o = OFF_W1 + (t * 2 + k) * CR
                return cp[:, o : o + CR]

            def w2_v(t):
                o = OFF_W2 + t * 64
                return cp[0:CR, o : o + 64]

            mask_v = cp[:, OFF_MASK : OFF_MASK + XW]
            bvb_v = cp[:, OFF_BVB : OFF_BVB + C]

            # x partial sums (for gamma*mean(x)) at s1p cols 128 + b*2 + k
            for b in range(B):
                for k in range(2):
                    cc = 128 + b * 2 + k
                    nc.vector.tensor_reduce(
                        out=s1p[:, cc : cc + 1],
                        in_=xt[:, b, k, 2 : 2 + NL],
                        axis=AX.X,
                        op=OP.add,
                    )

            # ---- phase A: q path per batch ------------------------------
            for b in range(B):
                q1_sb = []
                for mtile in range(2):
                    ps = qps.tile([128, XW], f32, tag="q1ps", name="q1ps")
                    for k in range(2):
                        for lo, hi in ((0, 512), (512, XW)):
                            nc.tensor.matmul(
                                ps[:, lo:hi],
                                wq_v(k)[:, mtile * 128 : (mtile + 1) * 128],
                                xt[:, b, k, lo:hi],
                                start=(k == 0),
                                stop=(k == 1),
                            )
                    q1 = qtmp.tile([128, XW], bf16, tag=f"q1_{mtile}", name=f"q1_{mtile}")
                    nc.scalar.activation(
                        out=q1, in_=ps, func=AF.Identity, bias=bq_v(mtile)
                    )
                    nc.vector.tensor_mul(q1, q1, mask_v)
                    q1_sb.append(q1)

                ps2 = q2ps.tile([CR, Q2W], f32, tag="q2ps", name="q2ps")
                for t in range(3):
                    for k in range(2):
                        st = t == 0 and k == 0
                        sp = t == 2 and k == 1
                        for lo, hi in ((0, 512), (512, Q2W)):
                            nc.tensor.matmul(
                                ps2[:, lo:hi],
                                w1_v(t, k),
                                q1_sb[k][:, lo + t : hi + t],
                                start=st,
                                stop=sp,
                            )
                q2 = qtmp.tile([CR, Q2W], bf16, tag="q2", name="q2")
                nc.scalar.activation(out=q2, in_=ps2, func=AF.Identity, bias=b1_v)
                nc.vector.tensor_mul(q2, q2, mask_v[:CR, 1 : 1 + Q2W])

                ps3 = q3ps.tile([2 * CR, NL], f32, tag="q3ps", name="q3ps")
                for t in range(3):
                    nc.tensor.matmul(
                        ps3,
                        w2_v(t),
                        q2[:, t : t + NL],
                        start=(t == 0),
                        stop=(t == 2),
                    )
                q3 = qtmp.tile([2 * CR, NL], f8, tag="q3", name="q3")
                nc.scalar.activation(
                    out=q3, in_=ps3, func=AF.Identity, bias=fpk[0 : 2 * CR, 3:4]
                )
                nc.sync.dma_start(
                    out=Q_all[CR * b : CR * (b + 1), :], in_=q3[0:CR, :]
                )
                nc.sync.dma_start(out=k_in[b], in_=q3[CR : 2 * CR, :])

            # single K AllGather for all 4 batches; high priority so the
            # scheduler keeps it AHEAD of the (bigger) V AllGather on the CC
            # queue -- phase C is gated on K
            with tc.high_priority(offset=1000):
                nc.gpsimd.collective_compute(
                    "AllGather",
                    OP.bypass,
                    replica_groups=RG,
                    ins=[k_in[:, :, :]],
                    outs=[k_out[:, :, :, :]],
                )

            # ---- phase B: value^T shards, fp8 ---------------------------
            for b in range(B):
                vstage = vst.tile([128, 4, C], f8, tag="vstage", name="vstage")
                for ms in range(4):
                    psv = vps.tile([128, C], f32, tag="vpsm", name="vpsm")
                    for k in range(2):
                        nc.tensor.matmul(
                            psv,
                            xt[:, b, k, 2 + ms * 128 : 2 + (ms + 1) * 128],
                            wv_v(k),
                            start=(k == 0),
                            stop=(k == 1),
                        )
                    nc.vector.tensor_add(vstage[:, ms, :], psv, bvb_v)
                nc.sync.dma_start(
                    out=v_in[b].rearrange("(ms p) c -> p ms c", p=128), in_=vstage
                )

            # assemble K_all from the gathered shards (per-b: the SBUF dst
            # must keep a single partition dim)
            for b in range(B):
                nc.sync.dma_start(
                    out=K_all[CR * b : CR * (b + 1), :].rearrange(
                        "c (g m) -> c g m", g=NCORES
                    ),
                    in_=k_out[:, b].rearrange("g c m -> c g m"),
                )

        # force the V AllGather BEHIND the K AllGather on the CC queue
        # (phase C is gated on K; the scheduler otherwise reorders them)
        with tc.tile_wait_until(0.06):
            nc.gpsimd.collective_compute(
                "AllGather",
                OP.bypass,
                replica_groups=RG,
                ins=[v_in[:, :, :]],
                outs=[v_out[:, :, :, :]],
            )

        # =================================================================
        # conv-x: ConvTranspose of the residual x, staged to SBUF (+bias).
        # Runs in the collective dead-zone; keeps the PE warm before C.
        # =================================================================
        NOUT = 9 * Wd  # 576 spatial outputs per (b, py, px)
        with tc.tile_pool(name="cvx", bufs=1, space="PSUM") as cvx:
            for py in range(2):
                for px in range(2):
                    psx = [
                        cvx.tile([128, NOUT], f32, tag=f"cvx{b}", name=f"cvx{b}")
                        for b in range(B)
                    ]
                    taps = [
                        (ky, kx, k)
                        for ky in (py, py + 2)
                        for kx in (px, px + 2)
                        for k in range(2)
                    ]
                    for ti, (ky, kx, k) in enumerate(taps):
                        ro = (py + ky) // 2 - py
                        ww = (px + kx) // 2 - 1
                        for b in range(B):
                            fp = fpx[b][k]
                            nc.tensor.matmul(
                                psx[b][:, 0:512],
                                wco_v(ky, kx, k),
                                fp[:, ro : ro + 8, 2 + ww : 66 + ww],
                                start=(ti == 0),
                                stop=(ti == len(taps) - 1),
                            )
                            nc.tensor.matmul(
                                psx[b][:, 512:NOUT],
                                wco_v(ky, kx, k),
                                fp[:, ro + 8, 2 + ww : 66 + ww],
                                start=(ti == 0),
                                stop=(ti == len(taps) - 1),
                            )
                    for b in range(B):
                        sv = stg[:, py, px, b]
                        pv = psx[b].rearrange("p (j w) -> p j w", w=Wd)
                        nc.scalar.activation(
                            out=sv[:, 1:9, :], in_=pv[:, 1:9, :],
                            func=AF.Identity, bias=bco_v,
                        )
                        nc.scalar.activation(
                            out=sv[:, 0:1, :], in_=pv[:, 0:1, :], func=AF.Copy,
                        )

        # =================================================================
        # phases C (QK + exp + denominators) and D (scale + fusion matmul)
        # =================================================================
        with tc.tile_pool(name="work", bufs=1) as work:
            # E in fp8e4 (exp bias -2 keeps E' <= ~130 < 448), stored as
            # m-tile PAIRS [128, 2, B, NL] for DoubleRow fusion matmuls
            e2 = [
                work.tile([128, 2, B, NL], f8, tag=f"e{t}", name=f"e{t}")
                for t in range(MT // 2)
            ]
            rb_sb = [
                work.tile([128, NL], bf16, tag=f"rb{mt}", name=f"rb{mt}")
                for mt in range(MT)
            ]

            with (
                tc.tile_pool(name="qk", bufs=2, space="PSUM") as qk,
                tc.tile_pool(name="sc", bufs=2) as sc,
            ):
                for mt in range(MT):
                    ps4 = qk.tile([128, B, NL], f32, tag="e4ps", name="e4ps")
                    for b in range(B):
                        nc.tensor.matmul(
                            ps4[:, b, :],
                            K_all[CR * b : CR * (b + 1), mt * 128 : (mt + 1) * 128],
                            Q_all[CR * b : CR * (b + 1), :],
                            start=True,
                            stop=True,
                            tile_position=(CR * b, 0),
                        )
                    ev = e2[mt // 2][:, mt % 2]  # [128, B, NL] view
                    # S1 partials: Scalar (per-b exp accum_out) for the
                    # tiles feeding the ARs (so AR inputs land with the exp,
                    # not at the DVE queue tail); DVE reduce for the middle
                    if mt % 8 < 3:
                        for b in range(B):
                            col = 4 * mt + b
                            nc.scalar.activation(
                                out=ev[:, b, :],
                                in_=ps4[:, b, :],
                                func=AF.Exp,
                                bias=nege2_v,
                                accum_out=s1p[:, col : col + 1],
                            )
                    else:
                        nc.scalar.activation(out=ev, in_=ps4, func=AF.Exp, bias=nege2_v)
                        nc.vector.tensor_reduce(
                            out=s1p[:, 4 * mt : 4 * mt + 4],
                            in_=ev,
                            axis=AX.X,
                            op=OP.add,
                        )
                    # S0 = sum_b E: add tree, mostly on GpSimd; late
                    # tiles on DVE so the GpSimd queue reaches the AR2
                    # trigger early (AR2 gates the mt>=16 A-scaling)
                    s0f = sc.tile([128, NL], f32, tag="s0f", name="s0f")
                    if mt % 4 == 0 or mt >= 20:
                        t2 = sc.tile([128, 2, NL], bf16, tag="t2", name="t2")
                        nc.vector.tensor_add(t2, ev[:, 0:2, :], ev[:, 2:4, :])
                        nc.vector.tensor_add(s0f, t2[:, 0, :], t2[:, 1, :])
                    else:
                        s01 = sc.tile([128, NL], bf16, tag="s01", name="s01")
                        s23 = sc.tile([128, NL], bf16, tag="s23", name="s23")
                        nc.gpsimd.tensor_add(s01, ev[:, 0, :], ev[:, 1, :])
                        nc.gpsimd.tensor_add(s23, ev[:, 2, :], ev[:, 3, :])
                        nc.gpsimd.tensor_add(s0f, s01, s23)
                    rf = sc.tile([128, NL], f32, tag="rf", name="rf")
                    nc.vector.reciprocal_approx_fast(out=rf, in_=s0f)
                    if mt % 2 == 0:
                        nc.scalar.copy(out=rb_sb[mt], in_=rf)
                    else:
                        nc.vector.tensor_copy(rb_sb[mt], rf)

                    if mt == MT // 2 - 1:
                        nc.sync.dma_start(out=ar1_in[:, :], in_=s1p[:, 0:64])
                        nc.gpsimd.collective_compute(
                            "AllReduce", OP.add, replica_groups=RG,
                            ins=[ar1_in[:, :]], outs=[ar1_out[:, :]],
                        )
                        nc.sync.dma_start(out=a1o, in_=ar1_out[:, :])
                        r1f = sc.tile([128, 64], f32, tag="r1f", name="r1f")
                        nc.vector.reciprocal_approx_fast(out=r1f, in_=a1o)
                        nc.vector.tensor_copy(r1a, r1f)

                # second AR half: S1 cols 64..128 plus the x sums
                nc.sync.dma_start(out=ar2_in[:, 0:64], in_=s1p[:, 64:128])
                nc.sync.dma_start(out=ar2_in[:, 64:72], in_=s1p[:, 128:136])
                nc.gpsimd.collective_compute(
                    "AllReduce", OP.add, replica_groups=RG,
                    ins=[ar2_in[:, :]], outs=[ar2_out[:, :]],
                )
                nc.sync.dma_start(out=a2o, in_=ar2_out[:, :])
                r2f = sc.tile([128, 64], f32, tag="r2f", name="r2f")
                nc.vector.reciprocal_approx_fast(out=r2f, in_=a2o[:, 0:64])
                nc.vector.tensor_copy(r1b, r2f)

                # g_bcast[p, b] = gamma * mean(x[b])
                xps = sc.tile([1, 8], f32, tag="xps", name="xps")
                nc.gpsimd.tensor_reduce(
                    out=xps, in_=a2o[:, 64:72], axis=AX.C, op=OP.add
                )
                xv = xps.rearrange("p (b k) -> p b k", b=B)
                g0 = sc.tile([1, B], f32, tag="g0", name="g0")
                nc.vector.tensor_add(g0, xv[:, :, 0], xv[:, :, 1])
                nc.vector.tensor_scalar(
                    out=g0,
                    in0=g0,
                    scalar1=gm_v,
                    scalar2=float(4.0 / (C * HW)),
                    op0=OP.mult,
                    op1=OP.mult,
                )
                nc.sync.dma_start(out=g_dram[:, :], in_=g0)
                nc.sync.dma_start(
                    out=g_bcast,
                    in_=bass.AP(
                        tensor=g_dram.tensor,
                        offset=g_dram.offset,
                        ap=[[0, 128], [1, B]],
                    ),
                )

            # raw fusion in fp8 (|fusion| ~ 13 << 448); gamma*mean(x) is
            # applied in the phase-E epilogue (convT is linear, g is a
            # per-batch scalar)
            ff8 = [
                work.tile([128, 2, 10, ROWW], f8, tag=f"ff8{b}", name=f"ff8{b}")
                for b in range(B)
            ]
            for b in range(B):
                nc.gpsimd.memset(ff8[b], 0.0)

            # ---- phase D: A = E*(1/S0 + 1/S1) in place; fusion matmuls --
            with (
                tc.tile_pool(name="fus", bufs=1, space="PSUM") as fus,
                tc.tile_pool(name="vtp", bufs=4) as vtp,
            ):
                fusion_ps = [
                    [
                        fus.tile([128, NL], f32, tag=f"f{b}_{ch}", name=f"f{b}_{ch}")
                        for ch in range(2)
                    ]
                    for b in range(B)
                ]
                NP = MT // 2
                for t in range(NP):
                    g = t // 2
                    ml = (t % 2) * 256
                    vt8 = vtp.tile([128, 2, B, C], f8, tag="vt8", name="vt8")
                    for b in range(B):
                        nc.sync.dma_start(
                            out=vt8[:, :, b, :],
                            in_=v_out[g, b, ml : ml + 256, :].rearrange(
                                "(two p) c -> p two c", p=128
                            ),
                        )
                    et = e2[t]
                    for par in range(2):
                        mt = 2 * t + par
                        r1h = r1a if mt < 16 else r1b
                        cb = (4 * mt) % 64
                        for b in range(B):
                            nc.vector.scalar_tensor_tensor(
                                out=et[:, par, b, :],
                                in0=rb_sb[mt],
                                scalar=r1h[:, cb + b : cb + b + 1],
                                in1=et[:, par, b, :],
                                op0=OP.add,
                                op1=OP.mult,
                            )
                    for b in range(B):
                        for ch in range(2):
                            nc.tensor.matmul(
                                fusion_ps[b][ch],
                                vt8[:, :, b, ch * 128 : (ch + 1) * 128],
                                et[:, :, b, :],
                                start=(t == 0),
                                stop=(t == NP - 1),
                                perf_mode=mybir.MatmulPerfMode.DoubleRow,
                            )

                # ---- stage raw fusion to fp8 conv layout ----------------
                for b in range(B):
                    for ch in range(2):
                        # scale by 1/4: TRN fp8e4 max-normal is 240 and
                        # |fusion| reaches ~275; the epilogue g absorbs the 4x
                        nc.scalar.activation(
                            out=ff8[b][:, ch, 1:9, 2:66],
                            in_=fusion_ps[b][ch].rearrange("p (r w) -> p r w", w=Wd),
                            func=AF.Copy,
                            scale=0.25,
                        )

        # =================================================================
        # phase E: ConvTranspose2d of the fusion branch (fp8 DoubleRow over
        # the two c-chunks), epilogue out = g_b * conv_f + staged conv_x
        # =================================================================
        with (
            tc.tile_pool(name="ostp", bufs=2) as ostp,
            tc.tile_pool(name="cps", bufs=1, space="PSUM") as cps,
        ):
            for py in range(2):
                ost = ostp.tile([128, B, 9, 2 * Wd], bf16, tag="ost", name="ost")
                for px in range(2):
                    pss = [
                        cps.tile([128, NOUT], f32, tag=f"cps{b}", name=f"cps{b}")
                        for b in range(B)
                    ]
                    taps = [
                        (ky, kx)
                        for ky in (py, py + 2)
                        for kx in (px, px + 2)
                    ]
                    for ti, (ky, kx) in enumerate(taps):
                        ro = (py + ky) // 2 - py
                        ww = (px + kx) // 2 - 1
                        for b in range(B):
                            fp = ff8[b]
                            nc.tensor.matmul(
                                pss[b][:, 0:512],
                                wco_pair(ky, kx),
                                fp[:, :, ro : ro + 8, 2 + ww : 66 + ww],
                                start=(ti == 0),
                                stop=(ti == len(taps) - 1),
                                perf_mode=mybir.MatmulPerfMode.DoubleRow,
                            )
                            nc.tensor.matmul(
                                pss[b][:, 512:NOUT],
                                wco_pair(ky, kx),
                                fp[:, :, ro + 8, 2 + ww : 66 + ww],
                                start=(ti == 0),
                                stop=(ti == len(taps) - 1),
                                perf_mode=mybir.MatmulPerfMode.DoubleRow,
                            )
                    for b in range(B):
                        ov = ost[:, b].rearrange("p j (w q) -> p j w q", q=2)[
                            :, :, :, px
                        ]
                        psv = pss[b].rearrange("p (j w) -> p j w", w=Wd)
                        nc.vector.scalar_tensor_tensor(
                            out=ov,
                            in0=psv,
                            scalar=g_bcast[:, b : b + 1],
                            in1=stg[:, py, px, b],
                            op0=OP.mult,
                            op1=OP.add,
                        )
                for b in range(B):
                    nc.sync.dma_start(
                        out=out_p[b].rearrange("c (j t) w -> c j t w", t=2)[
                            :, :, 1 - py, :
                        ],
                        in_=ost[:, b],
                    )

    nc.finalize()
    return nc


# ---------------------------------------------------------------------------
# host side
# ---------------------------------------------------------------------------
def _host_prep(x, wq, bq, wv, bv, w_adj1, b_adj1, w_adj2, b_adj2, gamma, w_co, b_co):
    import ml_dtypes

    bf16 = ml_dtypes.bfloat16
    x = np.asarray(x, np.float32).reshape(B, C, HW)
    xpad = np.zeros((B, C, HW + 4), np.float32)
    xpad[:, :, 2 : 2 + HW] = x

    wqT = np.ascontiguousarray(np.asarray(wq, np.float32).T)  # [C, C]
    wvT = np.ascontiguousarray(np.asarray(wv, np.float32).T)

    # grouped conv -> block-diagonal [3, 256, 32]
    w1 = np.zeros((3, C, CR), np.float32)
    wa1 = np.asarray(w_adj1, np.float32)  # [32, 8, 3]
    for g in range(CR):
        w1[:, g * 8 : (g + 1) * 8, g] = wa1[g].T  # [8,3] -> [3,8]

    # conv2 with output channels permuted to [query(32) | key(32)]
    wa2 = np.asarray(w_adj2, np.float32)  # [64, 32, 3]
    perm = np.concatenate([np.arange(0, 64, 2), np.arange(1, 64, 2)])
    w2 = np.ascontiguousarray(wa2[perm].transpose(2, 1, 0))  # [3, 32, 64]
    b2p = np.asarray(b_adj2, np.float32)[perm]

    # convT weights: flip, swap I/O -> [ky, kx, c_in, c_out] -> [32,128,128]
    wt = np.flip(np.asarray(w_co, np.float32), (2, 3)).transpose(1, 0, 2, 3)
    wco = np.ascontiguousarray(
        wt.transpose(2, 3, 1, 0).reshape(4, 4, 2, 128, 128).reshape(32, 128, 128)
    ).astype(bf16)

    # const pack (mask differs per core; rest shared)
    cbase = np.zeros((128, CPCOLS), np.float32)
    for k in range(2):
        cbase[:, OFF_WQ + k * 256 : OFF_WQ + (k + 1) * 256] = wqT[
            k * 128 : (k + 1) * 128, :
        ]
        cbase[:, OFF_WV + k * 256 : OFF_WV + (k + 1) * 256] = wvT[
            k * 128 : (k + 1) * 128, :
        ]
    for t in range(3):
        for k in range(2):
            o = OFF_W1 + (t * 2 + k) * CR
            cbase[:, o : o + CR] = w1[t, k * 128 : (k + 1) * 128, :]
        cbase[0:CR, OFF_W2 + t * 64 : OFF_W2 + (t + 1) * 64] = w2[t]
    cbase[:, OFF_BVB : OFF_BVB + C] = np.asarray(bv, np.float32)[None, :]

    # f32 pack: bq k0/k1, b1, b2(perm), bco, gamma
    fpack = np.zeros((128, 8), np.float32)
    bqf = np.asarray(bq, np.float32)
    fpack[:, 0] = bqf[0:128]
    fpack[:, 1] = bqf[128:256]
    fpack[0:CR, 2] = np.asarray(b_adj1, np.float32)
    fpack[0 : 2 * CR, 3] = b2p
    fpack[:, 4] = np.asarray(b_co, np.float32)
    fpack[0, 5] = np.asarray(gamma, np.float32).reshape(-1)[0]
    fpack[:, 6] = -2.0
    fpack = np.ascontiguousarray(fpack)

    in_maps = []
    for i in range(NCORES):
        n0 = i * NL
        xsl = xpad[:, :, n0 : n0 + XW]  # [B, C, XW]
        xpk = np.ascontiguousarray(
            xsl.reshape(B, 2, 128, XW).transpose(2, 0, 1, 3).astype(bf16)
        )
        j = np.arange(XW)
        valid = ((n0 - 2 + j) >= 0) & ((n0 - 2 + j) < HW)
        cpk = cbase.copy()
        cpk[:, OFF_MASK : OFF_MASK + XW] = valid.astype(np.float32)[None, :]
        in_maps.append(
            dict(
                cpack=np.ascontiguousarray(cpk.astype(bf16)),
                fpack=fpack,
                xpack=xpk,
                wco=wco,
            )
        )
    return in_maps


def _stitch(outs):
    full = np.zeros((B, C // 2, 2 * H, 2 * Wd), np.float32)
    for i in range(NCORES):
        y0 = 16 * i - 1
        lo = max(0, y0)
        hi = min(2 * H, y0 + OUTROWS)
        full[:, :, lo:hi, :] += np.asarray(
            outs[i][:, :, lo - y0 : hi - y0, :], np.float32
        )
    return full


def _get_nc():
    if "nc" not in _CACHE:
        _CACHE["nc"] = build_module()
    return _CACHE["nc"]


def run_spmd(in_maps, trace=False, **kw):
    from concourse.bass_utils import run_bass_kernel_spmd

    nc = _get_nc()
    return run_bass_kernel_spmd(
        nc, in_maps, core_ids=list(range(NCORES)), trace=trace, **kw
    )


def kernel(x, wq, bq, wv, bv, w_adj1, b_adj1, w_adj2, b_adj2, gamma, w_co, b_co):
    in_maps = _host_prep(
        x, wq, bq, wv, bv, w_adj1, b_adj1, w_adj2, b_adj2, gamma, w_co, b_co
    )
    res = run_spmd(in_maps)
    full = _stitch([r["out"] for r in res.results])
    # slab rows 0,1 carry no bias (the neighbor's rows complete them);
    # global row 0 has no neighbor, so add the bias here.
    full[:, :, 0, :] += np.asarray(b_co, np.float32)[None, :, None]
    return full.astype(np.float32)


# revision 36
# speedup vs baseline: 1.0214x; 1.0118x over previous
"""BidirectionalAttention Trainium2 Bass kernel — 8-core SPMD, v2.

Decomposition (same math as the verified baseline):
  q path : 1x1 conv (matmul) -> grouped conv1d k=3 -> conv1d k=3
  attn   : E = exp(q^T k); both softmaxes share one exp:
             attn_f + attn_b = E * (1/S0[n,m] + 1/S1[b,m])
             S0 = sum_b E  (batch softmax denom, axis=0)
             S1 = sum_n E  (row softmax denom, axis=1) -> two AllReduces
  fusion = value @ (attn_f+attn_b)^T scaled by gamma*mean(x_b), + x
  ConvTranspose2d(k=4,s=2,p=1) via the 4-subkernel parity decomposition,
  18-row output slabs with additive 2-row seams stitched on the host.

Performance structure (471us baseline -> ~352us):
  - K/Q/V and E in fp8e4 (TRN f8 max-normal is 240: exp carries a -2 bias
    so E' <= ~130; raw fusion is staged at 1/4 scale).  The attention
    branch output is scaled by gamma*mean(x) ~ 1e-4, so fp8 there moves
    the final output by ~1e-4 relative (verified by amplifying gamma
    3000x: fusion-dominant rel err ~5e-2, i.e. fp8-grade fidelity).
  - Warm-up 128-byte AllGather at t=0 absorbs the ~35us cross-core
    NEFF-start skew under phase A (the ncfw entry barrier otherwise
    stalls the first real collective).
  - One K AllGather (64KB) for all 4 batches; the 512KB V AllGather is
    pinned BEHIND it on the CC queue via tile_wait_until (phase C is
    gated on K; the scheduler otherwise reorders them).
  - ConvTranspose split by linearity: convT(g*s*fusion + x) =
    g*s*convT(fusion) + convT(x).  convT(x) runs in the collective
    dead-zone before phase C (PE stays warm) into an SBUF stage that
    also carries the bias; convT(fusion) uses fp8 DoubleRow over the
    two c-chunk k-tiles in phase E; epilogue = one scalar_tensor_tensor
    per (py,px,b): out = g_b * conv_f + stage.
  - Phase C: S1 partials via per-batch exp accum_out on Scalar for
    3-of-8 tiles, merged exp + DVE reduce otherwise (queue balance);
    S0 add-tree mostly on GpSimd; 1/S0 cached bf16 for phase D.
  - Phase D: A = (1/S0 + 1/S1) * E' in-place as one scalar_tensor_tensor
    per (m-pair, par, b); fusion matmuls are fp8 DoubleRow over m-tile
    pairs (rhs [128, 2, 512]), halving PE stream time.
  - Coalesced DMAs (one const pack, one x pack, per-b V stages, one wco
    load, per-b output DMAs); bf16 output, stitched on the host.
"""

import numpy as np

B = 4
C = 256
H = 64
Wd = 64
HW = H * Wd            # 4096
CR = 32                # C // 8
NCORES = 8
NL = HW // NCORES      # 512 owned attention rows (n) per core
HL = H // NCORES       # 8 owned image rows per core
MT = HW // 128         # 32 m-tiles of 128
XW = NL + 4            # x slab width (n halo +-2 for the two k=3 convs)
Q2W = NL + 2           # q2 width (halo +-1 for conv2)
ROWW = 68              # fusion_pad row width: [0,1]=zero, 2..65 data, [66,67]=zero
OUTROWS = 2 * HL + 2   # 18 output rows per core (2-row overlaps, host-stitched)

# const-pack column offsets (bf16 elements)
OFF_WQ = 0             # [2, 256]
OFF_WV = 512           # [2, 256]
OFF_W1 = 1024          # [3, 2, 32]
OFF_MASK = 1216        # [516]
OFF_BVB = 1732         # [256]
OFF_W2 = 1988          # rows 0:32, [3, 64]
CPCOLS = 2180

_CACHE = {}


# ---------------------------------------------------------------------------
# device module
# ---------------------------------------------------------------------------
def build_module():
    from contextlib import ExitStack

    import concourse.bass as bass
    import concourse.mybir as mybir
    from concourse import bacc
    from concourse.tile import TileContext

    f32 = mybir.dt.float32
    bf16 = mybir.dt.bfloat16
    f8 = mybir.dt.float8e4
    AF = mybir.ActivationFunctionType
    OP = mybir.AluOpType
    AX = mybir.AxisListType

    nc = bacc.Bacc(num_devices=NCORES)
    RG = [list(range(NCORES))]

    # ---- parameters (per-core) -------------------------------------------
    cpack_p = nc.declare_dram_parameter("cpack", [128, CPCOLS], bf16, isOutput=False)
    fpack_p = nc.declare_dram_parameter("fpack", [128, 8], f32, isOutput=False)
    xpack_p = nc.declare_dram_parameter("xpack", [128, B, 2, XW], bf16, isOutput=False)
    wco_p = nc.declare_dram_parameter("wco", [32, 128, 128], bf16, isOutput=False)
    out_p = nc.declare_dram_parameter(
        "out", [B, C // 2, OUTROWS, 2 * Wd], bf16, isOutput=True
    )

    with TileContext(nc) as tc, ExitStack() as ctx:
        # ---- long-lived pools -------------------------------------------
        const = ctx.enter_context(tc.tile_pool(name="const", bufs=1))
        xpool = ctx.enter_context(tc.tile_pool(name="xp", bufs=1))
        qkv = ctx.enter_context(tc.tile_pool(name="qkv", bufs=1))
        fpool = ctx.enter_context(tc.tile_pool(name="fp", bufs=1))
        dram = ctx.enter_context(tc.tile_pool(name="dram", bufs=1, space="DRAM"))

        # ---- DRAM bounce buffers ----------------------------------------
        k_in = dram.tile([B, CR, NL], f8, tag="k_in", name="k_in")
        k_out = dram.tile(
            [NCORES, B, CR, NL], f8, tag="k_out", name="k_out"
        )
        v_in = dram.tile([B, NL, C], f8, tag="v_in", name="v_in")
        v_out = dram.tile(
            [NCORES, B, NL, C], f8, tag="v_out", name="v_out"
        )
        ar1_in = dram.tile([128, 64], f32, tag="ar1_in", name="ar1_in")
        ar1_out = dram.tile(
            [128, 64], f32, tag="ar1_out", name="ar1_out"
        )
        ar2_in = dram.tile([128, 72], f32, tag="ar2_in", name="ar2_in")
        ar2_out = dram.tile(
            [128, 72], f32, tag="ar2_out", name="ar2_out"
        )
        g_dram = dram.tile([1, B], f32, tag="g_dram", name="g_dram")
        warm_in = dram.tile([1, 4], f32, tag="warm_in", name="warm_in")
        warm_out = dram.tile([NCORES, 4], f32, tag="warm_out", name="warm_out")

        # warm-up rendezvous: absorb the cross-core NEFF-start skew under
        # phase A instead of paying it at the first real collective
        with tc.high_priority():
            nc.gpsimd.collective_compute(
                "AllGather", OP.bypass, replica_groups=RG,
                ins=[warm_in[:, :]], outs=[warm_out[:, :]],
            )

        # ---- persistent SBUF state --------------------------------------
        fpk = const.tile([128, 8], f32, tag="fpk", name="fpk")
        nc.sync.dma_start(out=fpk, in_=fpack_p[:, :])
        xt = xpool.tile([128, B, 2, XW], bf16, tag="xt", name="xt")
        nc.sync.dma_start(out=xt, in_=xpack_p[:, :, :, :])

        s1p = qkv.tile([128, 136], f32, tag="s1p", name="s1p")
        Q_all = qkv.tile([128, NL], f8, tag="Q", name="Q")
        K_all = qkv.tile([128, HW], f8, tag="K", name="K")
        r1a = qkv.tile([128, 64], bf16, tag="r1a", name="r1a")  # 1/S1, mt<16
        r1b = qkv.tile([128, 64], bf16, tag="r1b", name="r1b")  # 1/S1, mt>=16
        g_bcast = qkv.tile([128, B], f32, tag="gbc", name="gbc")
        a1o = qkv.tile([128, 64], f32, tag="a1o", name="a1o")
        a2o = qkv.tile([128, 72], f32, tag="a2o", name="a2o")

        wt = const.tile([128, 32, 128], bf16, tag="wt", name="wt")
        nc.sync.dma_start(out=wt, in_=wco_p.rearrange("t p co -> p t co"))

        def wco_v(ky, kx, k):
            return wt[:, ky * 8 + kx * 2 + k, :]

        wt8 = const.tile([128, 32, 128], f8, tag="wt8", name="wt8")
        nc.scalar.copy(out=wt8, in_=wt)

        def wco_pair(ky, kx):
            return wt8[:, ky * 8 + kx * 2 : ky * 8 + kx * 2 + 2, :]

        # x in ConvT layout (halo rows/cols zero) and the staged convT(x)+bias
        fpx = [
            [
                fpool.tile([128, 10, ROWW], bf16, tag=f"fpx{b}_{ch}", name=f"fpx{b}_{ch}")
                for ch in range(2)
            ]
            for b in range(B)
        ]
        stg = fpool.tile([128, 2, 2, B, 9, Wd], bf16, tag="stg", name="stg")
        for b in range(B):
            for ch in range(2):
                nc.gpsimd.memset(fpx[b][ch], 0.0)
                nc.scalar.copy(
                    out=fpx[b][ch][:, 1:9, 2:66],
                    in_=xt[:, b, ch, 2 : 2 + NL].rearrange("p (r w) -> p r w", w=Wd),
                )

        def bq_v(k):
            return fpk[:, k : k + 1]

        b1_v = fpk[0:CR, 2:3]
        b2q_v = fpk[0:CR, 3:4]
        b2k_v = fpk[CR : 2 * CR, 3:4]
        bco_v = fpk[:, 4:5]
        gm_v = fpk[0:1, 5:6]
        nege2_v = fpk[:, 6:7]  # -2.0 exp bias (fp8 range)

        # =================================================================
        # phases A (q path) + B (value) under the scoped const pack
        # =================================================================
        with (
            tc.tile_pool(name="cpA", bufs=1) as cpA,
            tc.tile_pool(name="qtmp", bufs=2) as qtmp,
            tc.tile_pool(name="qps", bufs=2, space="PSUM") as qps,
            tc.tile_pool(name="q2ps", bufs=1, space="PSUM") as q2ps,
            tc.tile_pool(name="q3ps", bufs=1, space="PSUM") as q3ps,
            tc.tile_pool(name="vps", bufs=1, space="PSUM") as vps,
            tc.tile_pool(name="vst", bufs=2) as vst,
        ):
            cp = cpA.tile([128, CPCOLS], bf16, tag="cp", name="cp")
            nc.sync.dma_start(out=cp, in_=cpack_p[:, :])

            def wq_v(k):
                return cp[:, OFF_WQ + k * 256 : OFF_WQ + (k + 1) * 256]

            def wv_v(k):
                return cp[:, OFF_WV + k * 256 : OFF_WV + (k + 1) * 256]

            def w1_v(t, k):
                o = OFF_W1 + (t * 2 + k) * CR
                return cp[:, o : o + CR]

            def w2_v(t):
                o = OFF_W2 + t * 64
                return cp[0:CR, o : o + 64]

            mask_v = cp[:, OFF_MASK : OFF_MASK + XW]
            bvb_v = cp[:, OFF_BVB : OFF_BVB + C]

            # x partial sums (for gamma*mean(x)) at s1p cols 128 + b*2 + k
            for b in range(B):
                for k in range(2):
                    cc = 128 + b * 2 + k
                    nc.vector.tensor_reduce(
                        out=s1p[:, cc : cc + 1],
                        in_=xt[:, b, k, 2 : 2 + NL],
                        axis=AX.X,
                        op=OP.add,
                    )

            # ---- phase A: q path per batch ------------------------------
            for b in range(B):
                q1_sb = []
                for mtile in range(2):
                    ps = qps.tile([128, XW], f32, tag="q1ps", name="q1ps")
                    for k in range(2):
                        for lo, hi in ((0, 512), (512, XW)):
                            nc.tensor.matmul(
                                ps[:, lo:hi],
                                wq_v(k)[:, mtile * 128 : (mtile + 1) * 128],
                                xt[:, b, k, lo:hi],
                                start=(k == 0),
                                stop=(k == 1),
                            )
                    q1 = qtmp.tile([128, XW], bf16, tag=f"q1_{mtile}", name=f"q1_{mtile}")
                    nc.scalar.activation(
                        out=q1, in_=ps, func=AF.Identity, bias=bq_v(mtile)
                    )
                    nc.vector.tensor_mul(q1, q1, mask_v)
                    q1_sb.append(q1)

                ps2 = q2ps.tile([CR, Q2W], f32, tag="q2ps", name="q2ps")
                for t in range(3):
                    for k in range(2):
                        st = t == 0 and k == 0
                        sp = t == 2 and k == 1
                        for lo, hi in ((0, 512), (512, Q2W)):
                            nc.tensor.matmul(
                                ps2[:, lo:hi],
                                w1_v(t, k),
                                q1_sb[k][:, lo + t : hi + t],
                                start=st,
                                stop=sp,
                            )
                q2 = qtmp.tile([CR, Q2W], bf16, tag="q2", name="q2")
                nc.scalar.activation(out=q2, in_=ps2, func=AF.Identity, bias=b1_v)
                nc.vector.tensor_mul(q2, q2, mask_v[:CR, 1 : 1 + Q2W])

                ps3 = q3ps.tile([2 * CR, NL], f32, tag="q3ps", name="q3ps")
                for t in range(3):
                    nc.tensor.matmul(
                        ps3,
                        w2_v(t),
                        q2[:, t : t + NL],
                        start=(t == 0),
                        stop=(t == 2),
                    )
                q3 = qtmp.tile([2 * CR, NL], f8, tag="q3", name="q3")
                nc.scalar.activation(
                    out=q3, in_=ps3, func=AF.Identity, bias=fpk[0 : 2 * CR, 3:4]
                )
                nc.sync.dma_start(
                    out=Q_all[CR * b : CR * (b + 1), :], in_=q3[0:CR, :]
                )
                nc.sync.dma_start(out=k_in[b], in_=q3[CR : 2 * CR, :])

            # single K AllGather for all 4 batches; high priority so the
            # scheduler keeps it AHEAD of the (bigger) V AllGather on the CC
            # queue -- phase C is gated on K
            with tc.high_priority(offset=1000):
                nc.gpsimd.collective_compute(
                    "AllGather",
                    OP.bypass,
                    replica_groups=RG,
                    ins=[k_in[:, :, :]],
                    outs=[k_out[:, :, :, :]],
                )

            # ---- phase B: value^T shards, fp8 ---------------------------
            for b in range(B):
                vstage = vst.tile([128, 4, C], f8, tag="vstage", name="vstage")
                for ms in range(4):
                    psv = vps.tile([128, C], f32, tag="vpsm", name="vpsm")
                    for k in range(2):
                        nc.tensor.matmul(
                            psv,
                            xt[:, b, k, 2 + ms * 128 : 2 + (ms + 1) * 128],
                            wv_v(k),
                            start=(k == 0),
                            stop=(k == 1),
                        )
                    nc.vector.tensor_add(vstage[:, ms, :], psv, bvb_v)
                nc.sync.dma_start(
                    out=v_in[b].rearrange("(ms p) c -> p ms c", p=128), in_=vstage
                )

            # assemble K_all from the gathered shards (per-b: the SBUF dst
            # must keep a single partition dim)
            for b in range(B):
                nc.sync.dma_start(
                    out=K_all[CR * b : CR * (b + 1), :].rearrange(
                        "c (g m) -> c g m", g=NCORES
                    ),
                    in_=k_out[:, b].rearrange("g c m -> c g m"),
                )

        # force the V AllGather BEHIND the K AllGather on the CC queue
        # (phase C is gated on K; the scheduler otherwise reorders them)
        with tc.tile_wait_until(0.06):
            nc.gpsimd.collective_compute(
                "AllGather",
                OP.bypass,
                replica_groups=RG,
                ins=[v_in[:, :, :]],
                outs=[v_out[:, :, :, :]],
            )

        # =================================================================
        # conv-x: ConvTranspose of the residual x, staged to SBUF (+bias).
        # Runs in the collective dead-zone; keeps the PE warm before C.
        # =================================================================
        NOUT = 9 * Wd  # 576 spatial outputs per (b, py, px)
        with tc.tile_pool(name="cvx", bufs=1, space="PSUM") as cvx:
            for py in range(2):
                for px in range(2):
                    psx = [
                        cvx.tile([128, NOUT], f32, tag=f"cvx{b}", name=f"cvx{b}")
                        for b in range(B)
                    ]
                    taps = [
                        (ky, kx, k)
                        for ky in (py, py + 2)
                        for kx in (px, px + 2)
                        for k in range(2)
                    ]
                    for ti, (ky, kx, k) in enumerate(taps):
                        ro = (py + ky) // 2 - py
                        ww = (px + kx) // 2 - 1
                        for b in range(B):
                            fp = fpx[b][k]
                            nc.tensor.matmul(
                                psx[b][:, 0:512],
                                wco_v(ky, kx, k),
                                fp[:, ro : ro + 8, 2 + ww : 66 + ww],
                                start=(ti == 0),
                                stop=(ti == len(taps) - 1),
                            )
                            nc.tensor.matmul(
                                psx[b][:, 512:NOUT],
                                wco_v(ky, kx, k),
                                fp[:, ro + 8, 2 + ww : 66 + ww],
                                start=(ti == 0),
                                stop=(ti == len(taps) - 1),
                            )
                    for b in range(B):
                        sv = stg[:, py, px, b]
                        pv = psx[b].rearrange("p (j w) -> p j w", w=Wd)
                        nc.scalar.activation(
                            out=sv[:, 1:9, :], in_=pv[:, 1:9, :],
                            func=AF.Identity, bias=bco_v,
                        )
                        nc.scalar.activation(
                            out=sv[:, 0:1, :], in_=pv[:, 0:1, :], func=AF.Copy,
                        )

        # =================================================================
        # phases C (QK + exp + denominators) and D (scale + fusion matmul)
        # =================================================================
        with tc.tile_pool(name="work", bufs=1) as work:
            # E in fp8e4 (exp bias -2 keeps E' <= ~130 < 448), stored as
            # m-tile PAIRS [128, 2, B, NL] for DoubleRow fusion matmuls
            e2 = [
                work.tile([128, 2, B, NL], f8, tag=f"e{t}", name=f"e{t}")
                for t in range(MT // 2)
            ]
            rb_sb = [
                work.tile([128, NL], bf16, tag=f"rb{mt}", name=f"rb{mt}")
                for mt in range(MT)
            ]

            with (
                tc.tile_pool(name="qk", bufs=2, space="PSUM") as qk,
                tc.tile_pool(name="sc", bufs=2) as sc,
            ):
                for mt in range(MT):
                    ps4 = qk.tile([128, B, NL], f32, tag="e4ps", name="e4ps")
                    for b in range(B):
                        nc.tensor.matmul(
                            ps4[:, b, :],
                            K_all[CR * b : CR * (b + 1), mt * 128 : (mt + 1) * 128],
                            Q_all[CR * b : CR * (b + 1), :],
                            start=True,
                            stop=True,
                            tile_position=(CR * b, 0),
                        )
                    ev = e2[mt // 2][:, mt % 2]  # [128, B, NL] view
                    # S1 partials: Scalar (per-b exp accum_out) for the
                    # tiles feeding the ARs (so AR inputs land with the exp,
                    # not at the DVE queue tail); DVE reduce for the middle
                    if mt % 8 < 3 or mt >= 27:
                        for b in range(B):
                            col = 4 * mt + b
                            nc.scalar.activation(
                                out=ev[:, b, :],
                                in_=ps4[:, b, :],
                                func=AF.Exp,
                                bias=nege2_v,
                                accum_out=s1p[:, col : col + 1],
                            )
                    else:
                        nc.scalar.activation(out=ev, in_=ps4, func=AF.Exp, bias=nege2_v)
                        nc.vector.tensor_reduce(
                            out=s1p[:, 4 * mt : 4 * mt + 4],
                            in_=ev,
                            axis=AX.X,
                            op=OP.add,
                        )
                    # S0 = sum_b E: add tree, mostly on GpSimd; late
                    # tiles on DVE so the GpSimd queue reaches the AR2
                    # trigger early (AR2 gates the mt>=16 A-scaling)
                    s0f = sc.tile([128, NL], f32, tag="s0f", name="s0f")
                    if mt % 4 == 0 or mt >= 20:
                        t2 = sc.tile([128, 2, NL], bf16, tag="t2", name="t2")
                        nc.vector.tensor_add(t2, ev[:, 0:2, :], ev[:, 2:4, :])
                        nc.vector.tensor_add(s0f, t2[:, 0, :], t2[:, 1, :])
                    else:
                        s01 = sc.tile([128, NL], bf16, tag="s01", name="s01")
                        s23 = sc.tile([128, NL], bf16, tag="s23", name="s23")
                        nc.gpsimd.tensor_add(s01, ev[:, 0, :], ev[:, 1, :])
                        nc.gpsimd.tensor_add(s23, ev[:, 2, :], ev[:, 3, :])
                        nc.gpsimd.tensor_add(s0f, s01, s23)
                    rf = sc.tile([128, NL], f32, tag="rf", name="rf")
                    nc.vector.reciprocal_approx_fast(out=rf, in_=s0f)
                    if mt % 2 == 0:
                        nc.scalar.copy(out=rb_sb[mt], in_=rf)
                    else:
                        nc.vector.tensor_copy(rb_sb[mt], rf)

                    if mt == MT // 2 - 1:
                        nc.sync.dma_start(out=ar1_in[:, :], in_=s1p[:, 0:64])
                        nc.gpsimd.collective_compute(
                            "AllReduce", OP.add, replica_groups=RG,
                            ins=[ar1_in[:, :]], outs=[ar1_out[:, :]],
                        )
                        nc.sync.dma_start(out=a1o, in_=ar1_out[:, :])
                        r1f = sc.tile([128, 64], f32, tag="r1f", name="r1f")
                        nc.vector.reciprocal_approx_fast(out=r1f, in_=a1o)
                        nc.vector.tensor_copy(r1a, r1f)

                # second AR half: S1 cols 64..128 plus the x sums
                nc.sync.dma_start(out=ar2_in[:, 0:64], in_=s1p[:, 64:128])
                nc.sync.dma_start(out=ar2_in[:, 64:72], in_=s1p[:, 128:136])
                nc.gpsimd.collective_compute(
                    "AllReduce", OP.add, replica_groups=RG,
                    ins=[ar2_in[:, :]], outs=[ar2_out[:, :]],
                )
                nc.sync.dma_start(out=a2o, in_=ar2_out[:, :])
                r2f = sc.tile([128, 64], f32, tag="r2f", name="r2f")
                nc.vector.reciprocal_approx_fast(out=r2f, in_=a2o[:, 0:64])
                nc.vector.tensor_copy(r1b, r2f)

                # g_bcast[p, b] = gamma * mean(x[b])
                xps = sc.tile([1, 8], f32, tag="xps", name="xps")
                nc.gpsimd.tensor_reduce(
                    out=xps, in_=a2o[:, 64:72], axis=AX.C, op=OP.add
                )
                xv = xps.rearrange("p (b k) -> p b k", b=B)
                g0 = sc.tile([1, B], f32, tag="g0", name="g0")
                nc.vector.tensor_add(g0, xv[:, :, 0], xv[:, :, 1])
                nc.vector.tensor_scalar(
                    out=g0,
                    in0=g0,
                    scalar1=gm_v,
                    scalar2=float(4.0 / (C * HW)),
                    op0=OP.mult,
                    op1=OP.mult,
                )
                nc.sync.dma_start(out=g_dram[:, :], in_=g0)
                nc.sync.dma_start(
                    out=g_bcast,
                    in_=bass.AP(
                        tensor=g_dram.tensor,
                        offset=g_dram.offset,
                        ap=[[0, 128], [1, B]],
                    ),
                )

            # raw fusion in fp8 (|fusion| ~ 13 << 448); gamma*mean(x) is
            # applied in the phase-E epilogue (convT is linear, g is a
            # per-batch scalar)
            ff8 = [
                work.tile([128, 2, 10, ROWW], f8, tag=f"ff8{b}", name=f"ff8{b}")
                for b in range(B)
            ]
            for b in range(B):
                nc.gpsimd.memset(ff8[b], 0.0)

            # ---- phase D: A = E*(1/S0 + 1/S1) in place; fusion matmuls --
            with (
                tc.tile_pool(name="fus", bufs=1, space="PSUM") as fus,
                tc.tile_pool(name="vtp", bufs=4) as vtp,
            ):
                fusion_ps = [
                    [
                        fus.tile([128, NL], f32, tag=f"f{b}_{ch}", name=f"f{b}_{ch}")
                        for ch in range(2)
                    ]
                    for b in range(B)
                ]
                NP = MT // 2
                for t in range(NP):
                    g = t // 2
                    ml = (t % 2) * 256
                    vt8 = vtp.tile([128, 2, B, C], f8, tag="vt8", name="vt8")
                    for b in range(B):
                        nc.sync.dma_start(
                            out=vt8[:, :, b, :],
                            in_=v_out[g, b, ml : ml + 256, :].rearrange(
                                "(two p) c -> p two c", p=128
                            ),
                        )
                    et = e2[t]
                    for par in range(2):
                        mt = 2 * t + par
                        r1h = r1a if mt < 16 else r1b
                        cb = (4 * mt) % 64
                        for b in range(B):
                            nc.vector.scalar_tensor_tensor(
                                out=et[:, par, b, :],
                                in0=rb_sb[mt],
                                scalar=r1h[:, cb + b : cb + b + 1],
                                in1=et[:, par, b, :],
                                op0=OP.add,
                                op1=OP.mult,
                            )
                    for b in range(B):
                        for ch in range(2):
                            nc.tensor.matmul(
                                fusion_ps[b][ch],
                                vt8[:, :, b, ch * 128 : (ch + 1) * 128],
                                et[:, :, b, :],
                                start=(t == 0),
                                stop=(t == NP - 1),
                                perf_mode=mybir.MatmulPerfMode.DoubleRow,
                            )

                # ---- stage raw fusion to fp8 conv layout ----------------
                for b in range(B):
                    for ch in range(2):
                        # scale by 1/4: TRN fp8e4 max-normal is 240 and
                        # |fusion| reaches ~275; the epilogue g absorbs the 4x
                        nc.scalar.activation(
                            out=ff8[b][:, ch, 1:9, 2:66],
                            in_=fusion_ps[b][ch].rearrange("p (r w) -> p r w", w=Wd),
                            func=AF.Copy,
                            scale=0.25,
                        )

        # =================================================================
        # phase E: ConvTranspose2d of the fusion branch (fp8 DoubleRow over
        # the two c-chunks), epilogue out = g_b * conv_f + staged conv_x
        # =================================================================
        with (
            tc.tile_pool(name="ostp", bufs=2) as ostp,
            tc.tile_pool(name="cps", bufs=1, space="PSUM") as cps,
        ):
            for py in range(2):
                ost = ostp.tile([128, B, 9, 2 * Wd], bf16, tag="ost", name="ost")
                for px in range(2):
                    pss = [
                        cps.tile([128, NOUT], f32, tag=f"cps{b}", name=f"cps{b}")
                        for b in range(B)
                    ]
                    taps = [
                        (ky, kx)
                        for ky in (py, py + 2)
                        for kx in (px, px + 2)
                    ]
                    for ti, (ky, kx) in enumerate(taps):
                        ro = (py + ky) // 2 - py
                        ww = (px + kx) // 2 - 1
                        for b in range(B):
                            fp = ff8[b]
                            nc.tensor.matmul(
                                pss[b][:, 0:512],
                                wco_pair(ky, kx),
                                fp[:, :, ro : ro + 8, 2 + ww : 66 + ww],
                                start=(ti == 0),
                                stop=(ti == len(taps) - 1),
                                perf_mode=mybir.MatmulPerfMode.DoubleRow,
                            )
                            nc.tensor.matmul(
                                pss[b][:, 512:NOUT],
                                wco_pair(ky, kx),
                                fp[:, :, ro + 8, 2 + ww : 66 + ww],
                                start=(ti == 0),
                                stop=(ti == len(taps) - 1),
                                perf_mode=mybir.MatmulPerfMode.DoubleRow,
                            )
                    for b in range(B):
                        ov = ost[:, b].rearrange("p j (w q) -> p j w q", q=2)[
                            :, :, :, px
                        ]
                        psv = pss[b].rearrange("p (j w) -> p j w", w=Wd)
                        nc.vector.scalar_tensor_tensor(
                            out=ov,
                            in0=psv,
                            scalar=g_bcast[:, b : b + 1],
                            in1=stg[:, py, px, b],
                            op0=OP.mult,
                            op1=OP.add,
                        )
                for b in range(B):
                    nc.sync.dma_start(
                        out=out_p[b].rearrange("c (j t) w -> c j t w", t=2)[
                            :, :, 1 - py, :
                        ],
                        in_=ost[:, b],
                    )

    nc.finalize()
    return nc


# ---------------------------------------------------------------------------
# host side
# ---------------------------------------------------------------------------
def _host_prep(x, wq, bq, wv, bv, w_adj1, b_adj1, w_adj2, b_adj2, gamma, w_co, b_co):
    import ml_dtypes

    bf16 = ml_dtypes.bfloat16
    x = np.asarray(x, np.float32).reshape(B, C, HW)
    xpad = np.zeros((B, C, HW + 4), np.float32)
    xpad[:, :, 2 : 2 + HW] = x

    wqT = np.ascontiguousarray(np.asarray(wq, np.float32).T)  # [C, C]
    wvT = np.ascontiguousarray(np.asarray(wv, np.float32).T)

    # grouped conv -> block-diagonal [3, 256, 32]
    w1 = np.zeros((3, C, CR), np.float32)
    wa1 = np.asarray(w_adj1, np.float32)  # [32, 8, 3]
    for g in range(CR):
        w1[:, g * 8 : (g + 1) * 8, g] = wa1[g].T  # [8,3] -> [3,8]

    # conv2 with output channels permuted to [query(32) | key(32)]
    wa2 = np.asarray(w_adj2, np.float32)  # [64, 32, 3]
    perm = np.concatenate([np.arange(0, 64, 2), np.arange(1, 64, 2)])
    w2 = np.ascontiguousarray(wa2[perm].transpose(2, 1, 0))  # [3, 32, 64]
    b2p = np.asarray(b_adj2, np.float32)[perm]

    # convT weights: flip, swap I/O -> [ky, kx, c_in, c_out] -> [32,128,128]
    wt = np.flip(np.asarray(w_co, np.float32), (2, 3)).transpose(1, 0, 2, 3)
    wco = np.ascontiguousarray(
        wt.transpose(2, 3, 1, 0).reshape(4, 4, 2, 128, 128).reshape(32, 128, 128)
    ).astype(bf16)

    # const pack (mask differs per core; rest shared)
    cbase = np.zeros((128, CPCOLS), np.float32)
    for k in range(2):
        cbase[:, OFF_WQ + k * 256 : OFF_WQ + (k + 1) * 256] = wqT[
            k * 128 : (k + 1) * 128, :
        ]
        cbase[:, OFF_WV + k * 256 : OFF_WV + (k + 1) * 256] = wvT[
            k * 128 : (k + 1) * 128, :
        ]
    for t in range(3):
        for k in range(2):
            o = OFF_W1 + (t * 2 + k) * CR
            cbase[:, o : o + CR] = w1[t, k * 128 : (k + 1) * 128, :]
        cbase[0:CR, OFF_W2 + t * 64 : OFF_W2 + (t + 1) * 64] = w2[t]
    cbase[:, OFF_BVB : OFF_BVB + C] = np.asarray(bv, np.float32)[None, :]

    # f32 pack: bq k0/k1, b1, b2(perm), bco, gamma
    fpack = np.zeros((128, 8), np.float32)
    bqf = np.asarray(bq, np.float32)
    fpack[:, 0] = bqf[0:128]
    fpack[:, 1] = bqf[128:256]
    fpack[0:CR, 2] = np.asarray(b_adj1, np.float32)
    fpack[0 : 2 * CR, 3] = b2p
    fpack[:, 4] = np.asarray(b_co, np.float32)
    fpack[0, 5] = np.asarray(gamma, np.float32).reshape(-1)[0]
    fpack[:, 6] = -2.0
    fpack = np.ascontiguousarray(fpack)

    in_maps = []
    for i in range(NCORES):
        n0 = i * NL
        xsl = xpad[:, :, n0 : n0 + XW]  # [B, C, XW]
        xpk = np.ascontiguousarray(
            xsl.reshape(B, 2, 128, XW).transpose(2, 0, 1, 3).astype(bf16)
        )
        j = np.arange(XW)
        valid = ((n0 - 2 + j) >= 0) & ((n0 - 2 + j) < HW)
        cpk = cbase.copy()
        cpk[:, OFF_MASK : OFF_MASK + XW] = valid.astype(np.float32)[None, :]
        in_maps.append(
            dict(
                cpack=np.ascontiguousarray(cpk.astype(bf16)),
                fpack=fpack,
                xpack=xpk,
                wco=wco,
            )
        )
    return in_maps


def _stitch(outs):
    full = np.zeros((B, C // 2, 2 * H, 2 * Wd), np.float32)
    for i in range(NCORES):
        y0 = 16 * i - 1
        lo = max(0, y0)
        hi = min(2 * H, y0 + OUTROWS)
        full[:, :, lo:hi, :] += np.asarray(
            outs[i][:, :, lo - y0 : hi - y0, :], np.float32
        )
    return full


def _get_nc():
    if "nc" not in _CACHE:
        _CACHE["nc"] = build_module()
    return _CACHE["nc"]


def run_spmd(in_maps, trace=False, **kw):
    from concourse.bass_utils import run_bass_kernel_spmd

    nc = _get_nc()
    return run_bass_kernel_spmd(
        nc, in_maps, core_ids=list(range(NCORES)), trace=trace, **kw
    )


def kernel(x, wq, bq, wv, bv, w_adj1, b_adj1, w_adj2, b_adj2, gamma, w_co, b_co):
    in_maps = _host_prep(
        x, wq, bq, wv, bv, w_adj1, b_adj1, w_adj2, b_adj2, gamma, w_co, b_co
    )
    res = run_spmd(in_maps)
    full = _stitch([r["out"] for r in res.results])
    # slab rows 0,1 carry no bias (the neighbor's rows complete them);
    # global row 0 has no neighbor, so add the bias here.
    full[:, :, 0, :] += np.asarray(b_co, np.float32)[None, :, None]
    return full.astype(np.float32)
